# revision 41
# baseline (speedup 1.0000x reference)
"""Trainium2 Bass kernel for the MLPSim adjacency-constructor problem.

Full shapes: spatial [4, 2048, 32], temporal [4, 288, 32], output
adj [4, 2336, 2336] f32 where adj = tanh(relu(blocks)):
  ss = tanh(m - m^T), m = nv1 @ nv2^T, nv_i = tanh(3*x@W_i^T)
  st = s1[n] + s2[t] + b_st ;  ts = s1t[t] + s2t[n] + b_ts
  tt = triu(temporal @ temporal^T)

Sharding: 8 cores = (batch b = c//2) x (row-half h = c%2); each core emits
1024 spatial + 144 temporal rows ([1168, 2336]) of one batch. Spatial
COLUMNS are rotated by -h*1024 on the host so each core's row-half sits at
columns 0:1024 (assembly un-rotates).

Device algebra (ACT-bound design, fp16 datapath):
  ss: tanh(relu(tanh(d))) ~= S*relu(tanh(C*d)), minimax S=0.7552623
      C=1.2825139 (maxerr 6.3e-3) -> ONE ACT tanh pass from PSUM + ONE
      DVE dual-op tensor_scalar (max 0, mult S) at 4x fp16 mode.
  Prep z = x@W packs BOTH spatial column-halves into one [128,1024] PSUM
  tile (cols 0:1024 in partitions 0:64, cols 1024:2048 in 64:128); the
  two partition groups use PE weight tiles (0,*) and (0,64) and their
  matmuls overlap on the array, and the nv tanh is 2 ACT passes at half
  the columns. Rp keeps the packed layout; zstep j reads rhs from
  partitions 64j:64j+64 with the weight tile at row 64j (lhsT always
  LtBuf[0:64] = [-nv2; nv1], loaded via explicit tile_position). d
  accumulates in K=64 fp16 matmuls; the PSUM rotation is 3-deep for
  s<6 and 4-deep after (qtt joins the cycle once the att pass drains
  it, emitted one chunk early so the s=6 gate clears in time) - with
  zstep s waiting zact[s-4] the PE fills each slot ~1.1us before ACT
  needs it and the zact phase runs gap-free; do NOT merge adjacent-slot
  zact pairs into one [128,2048] pass (holding two slots starves the PE
  and costs ~1.1us/chunk). prep/tt matmuls use fp16 hi/lo splitting
  (accumulating
  passes at 1 cyc/col), all hi passes first so the late-landing lo blob
  overlaps them. Broadcast/linear tiles (stb, s2tbF, biases as fp16
  hi/lo pairs) are host inputs; no PE broadcasts. Input DMA triggers
  issue from the Sync hwdge queue critical-consumer first, plus one on
  the idle Scalar queue (gpsimd software-DGE triggers measured ~3us
  issue latency - do not use). The st block is built by DVE bias
  pre-adds into one [128, 8*288] strip, tanh'd in three ACT passes,
  relu'd once, and stored with a single rearranged DMA. Temporal ts
  rows are one fused [128,2048] ACT pass in zact slack; both tt row
  groups share one ACT pass via a strided 2-bank PSUM AP. Semaphore
  waits are embedded in the consuming instruction (separate
  EVENT_SEMAPHORE ops cost ~100ns each on the queue). The tail splits
  the last zact into 512-col pieces so relu/store overlap it. All
  stores fp16; host assembles and upcasts.
  Runtime pitfalls baked in: fp32 matmuls run as 2 half-speed passes
  (fp32r is verifier-trapped; fp16 hi/lo instead); 1-partition matmuls
  need their own PSUM bank; same-engine RAW needs a semaphore even
  between adjacent queue entries (biases reconstruct vs first consumer
  raced without one); a DMA semaphore with multiple writers only
  supports all-or-nothing waits; fmap and weights must share the SB
  base partition (hence the LtBuf base-64 duplicate); tanh-degree-11
  polynomials on DVE lose to fp16 intermediate rounding (2.8e-2) - not
  a viable ACT offload.
"""

import numpy as np
from contextlib import ExitStack

import concourse.bass as bass
from concourse import mybir
from concourse.bass_utils import run_bass_kernel_spmd

AF = mybir.ActivationFunctionType
OP = mybir.AluOpType
F32 = mybir.dt.float32
F16 = mybir.dt.float16

B, N, T, D = 4, 2048, 288, 32
NS = N // 2          # 1024 spatial rows per core
TS = T // 2          # 144 temporal rows per core
NT = N + T           # 2336
ROWS = NS + TS       # 1168
N_CORES = 8
NCHUNK = NS // 128   # 8 spatial row-chunks

SS_S = 0.7552623    # tanh(relu(tanh(d))) ~= SS_S * relu(tanh(SS_C*d))
SS_C = 1.2825139

PREP_PASSES = 3      # hi@Whi, hi@Wlo, lo@Whi
GTT_PASSES = 3       # hi.hi, hi.lo, lo.hi

# blob1 [D, B1_W] fp16: prep inputs (W transposed pair, spatial hi/lo)
_B1_DEFS = (("W12T_hi", 2 * D), ("W12T_lo", 2 * D), ("spT_hi", N), ("spT_lo", N))
B1_SLICES = []
_c = 0
for _nm, _w in _B1_DEFS:
    B1_SLICES.append((_nm, _c, _c + _w))
    _c += _w
B1_W = _c

# tmblob [D, TM_W] fp16: temporal hi/lo for the tt matmuls
_TM_DEFS = (("tmT_hi", T), ("tmT_lo", T), ("tmrT_hi", TS), ("tmrT_lo", TS))
TM_SLICES = []
_c = 0
for _nm, _w in _TM_DEFS:
    TM_SLICES.append((_nm, _c, _c + _w))
    _c += _w
TM_W = _c

# blob2s [128, B2S_W] fp16: small early broadcast tiles
#   stb[p, t] = s2[t] + b_st, s2tbk1[t*8+b, c] = s2t[b*256+c],
#   biases packed as fp16 hi/lo pairs (DVE reconstructs f32 on device)
NBIAS = NCHUNK + 3
B2S_W = T + 256 + 2 * NBIAS


def build_program():
    nc = bass.Bass()
    inp = {}

    def di(name, shape, dt=F16):
        inp[name] = nc.declare_dram_parameter(name, list(shape), dt, isOutput=False)

    di("blob1", (D, B1_W))
    di("blob2s", (128, B2S_W))
    di("s2tbF", (128, N))          # host bcast: s2t + b_ts (rotated col order)
    di("tmblob", (D, TM_W))
    di("ttmask", (TS, T))
    out = nc.declare_dram_parameter("out", [ROWS, NT], F16, isOutput=True)

    ctx = ExitStack()
    _uid = [0]

    def sbuf(shape, dt=F16):
        _uid[0] += 1
        return ctx.enter_context(nc.sbuf_tensor(f"sb{_uid[0]}", shape, dt))

    def psum(shape):
        _uid[0] += 1
        return ctx.enter_context(nc.psum_tensor(f"ps{_uid[0]}", shape, F32))

    with ctx:
        blob1 = sbuf([D, B1_W])
        t_in = {nm: blob1[:, c0:c1] for nm, c0, c1 in B1_SLICES}
        tmblob = sbuf([D, TM_W])
        for nm, c0, c1 in TM_SLICES:
            t_in[nm] = tmblob[:, c0:c1]
        blob2s = sbuf([128, B2S_W])
        stb = blob2s[:, 0:T]
        s2tbk1 = blob2s[:, T:T + 256]
        bias_hi = blob2s[:, T + 256:T + 256 + NBIAS]
        bias_lo = blob2s[:, T + 256 + NBIAS:T + 256 + 2 * NBIAS]
        s2tbF = sbuf([128, N])
        dummy = sbuf([1, 16])
        biases = sbuf([128, NBIAS], F32)
        s1col = biases[:, 0:NCHUNK]
        s1tcol = biases[:, NCHUNK:NCHUNK + 2]
        s1tk1 = biases[:, NCHUNK + 2:NCHUNK + 3]
        mask0 = sbuf([128, T])
        mask1 = sbuf([TS - 128, T])
        # packed nv: partitions 0:64 = [nv1; nv2] for spatial cols 0:1024,
        # partitions 64:128 = same for cols 1024:2048
        Rp = sbuf([128, 1024])
        # lhsT: rows 0:32 = -nv2, rows 32:64 = nv1 (at the chunk's rows);
        # rows 64:128 duplicate rows 0:64 (fmap and weights must share the
        # SB base partition, so the j=1 weight tile needs a base-64 copy)
        LtBuf = sbuf([128, NS])
        stba = sbuf([128, NCHUNK * T])       # st strips: stb + s1col[i]
        stbv = sbuf([128, NCHUNK * T])       # tanh(st strips)
        outbufs = [sbuf([128, N]) for _ in range(4)]
        tob0 = sbuf([128, NT])               # temporal k=0 rows
        tsk1buf = sbuf([128, 256])           # temporal k=1 ts region, packed
        ttk1buf = sbuf([TS - 128, T])        # temporal k=1 tt region
        tttbufM = sbuf([128, 2 * T])         # tt tanh: k=0 | k=1 halves

        zps = [psum([128, 1024]) for _ in range(3)]   # 6 banks
        # gtt: k=0 rows in cols 0:T (bank 6), k=1 rows in 512:512+T (bank 7)
        qtt = psum([128, 1024])

        dmain = ctx.enter_context(nc.semaphore("dmain"))
        dmain2 = ctx.enter_context(nc.semaphore("dmain2"))
        dmain3 = ctx.enter_context(nc.semaphore("dmain3"))
        dmain4 = ctx.enter_context(nc.semaphore("dmain4"))
        dmain5 = ctx.enter_context(nc.semaphore("dmain5"))
        dmain6 = ctx.enter_context(nc.semaphore("dmain6"))
        pe_s = ctx.enter_context(nc.semaphore("pe_s"))
        act_s = ctx.enter_context(nc.semaphore("act_s"))
        dve_s = ctx.enter_context(nc.semaphore("dve_s"))
        douts = [ctx.enter_context(nc.semaphore(f"dout{k}")) for k in range(5)]
        SEM = {"pe": pe_s, "act": act_s, "dve": dve_s, "din": dmain,
               "din2": dmain2, "din3": dmain3, "din4": dmain4, "din5": dmain5,
               "din6": dmain6,
               "dout0": douts[0], "dout1": douts[1], "dout2": douts[2],
               "dout3": douts[3], "dout4": douts[4]}

        plan = {"sync": [], "tensor": [], "scalar": [], "vector": [], "gpsimd": []}
        cnt = {"pe": 0, "act": 0, "dve": 0, "din": 0, "din2": 0, "din3": 0,
               "din4": 0, "din5": 0, "din6": 0,
               "dout0": 0, "dout1": 0, "dout2": 0, "dout3": 0, "dout4": 0}

        def op(engine, waits, fn, inc=None, delta=None):
            plan[engine].append((waits or [], fn, inc))
            if inc:
                if delta is None:
                    delta = 16 if inc.startswith("d") and inc != "dve" else 1
                cnt[inc] += delta
                return cnt[inc]
            return None

        # ---------- input loads: sync hwdge queue, priority order ----------
        # blob1 splits at the hi/lo boundary: the lo passes run last on PE,
        # so prep starts ~1.4us earlier on just W + spT_hi
        B1H = 2 * (2 * D) + N
        op("sync", None, lambda: nc.sync.dma_start(out=blob1[:, 0:B1H], in_=inp["blob1"][:, 0:B1H]), "din", delta=16)
        din_b1 = cnt["din"]
        # blob2s rides the Scalar hwdge queue (idle until the table load),
        # landing ~2us earlier than as sync trigger #2
        op("scalar", None, lambda: nc.scalar.dma_start(out=blob2s[:], in_=inp["blob2s"][:]), "din3", delta=16)
        din_b2 = cnt["din3"]
        op("sync", None, lambda: nc.sync.dma_start(out=blob1[:, B1H:B1_W], in_=inp["blob1"][:, B1H:B1_W]), "din2", delta=16)
        din_b1lo = cnt["din2"]
        op("sync", None, lambda: nc.sync.dma_start(out=s2tbF[:], in_=inp["s2tbF"][:]), "din6", delta=16)
        din_sF = cnt["din6"]
        op("sync", None, lambda: nc.sync.dma_start(out=tmblob[:], in_=inp["tmblob"][:]), "din5", delta=16)
        din_tm = cnt["din5"]
        op("sync", None, lambda: nc.sync.dma_start(out=mask0[:], in_=inp["ttmask"][0:128, :]), "din4", delta=16)
        op("sync", None, lambda: nc.sync.dma_start(out=mask1[:], in_=inp["ttmask"][128:TS, :]), "din4", delta=16)
        din_masks = cnt["din4"]

        Whi, Wlo = t_in["W12T_hi"], t_in["W12T_lo"]
        mm = nc.tensor.matmul
        act_i = nc.scalar.activation

        def pe(waits, fn, inc=None):
            return op("tensor", waits, fn, inc)

        def act(waits, fn, inc=True):
            return op("scalar", waits, fn, "act" if inc else None)

        def dve(waits, fn, inc=True):
            return op("vector", waits, fn, "dve" if inc else None)

        # ---------- nv prep: z = x@W, hi/lo accumulate, 128-part packed ----
        # partition group g covers spatial cols [g*1024, (g+1)*1024) at psum
        # cols = local col; both groups' matmuls overlap on the PE array.
        # all hi passes (blob1a) first, lo passes (blob1b) last: the late
        # blob1b landing then overlaps the hi matmuls instead of stalling
        # each region; adjacent different-tile matmuls overlap on the array
        def prep_region(cc, g):
            sc = g * 1024 + cc
            return zps[0][64 * g:64 * (g + 1), cc:cc + 512], sc

        first = [True]
        for hi_t in (Whi, Wlo):
            for cc in (0, 512):
                for g in range(2):
                    dst, sc = prep_region(cc, g)
                    w = [("din", din_b1)] if first[0] else None
                    first[0] = False
                    pe(w, lambda dst=dst, sc=sc, hi_t=hi_t: mm(
                        dst, hi_t[:], t_in["spT_hi"][:, sc:sc + 512],
                        start=hi_t is Whi, stop=False))
        ga = gb = None
        for cc in (0, 512):
            for g in range(2):
                dst, sc = prep_region(cc, g)
                w = [("din2", din_b1lo)] if (cc == 0 and g == 0) else None
                gend = pe(w, lambda dst=dst, sc=sc: mm(
                    dst, Whi[:], t_in["spT_lo"][:, sc:sc + 512],
                    start=False, stop=True), "pe")
            if cc == 0:
                ga = gend
            else:
                gb = gend

        # ---------- ACT stream: warm, early fill, prep tanh, zacts ---------
        d_dum = dve(None, lambda: nc.vector.memset(dummy[:], 0.25))
        act([("dve", d_dum)], lambda: act_i(dummy[0:1, 8:16], dummy[0:1, 0:8], AF.Tanh),
            inc=False)  # warms the Tanh table during the input-DMA wait

        # biases arrive as fp16 hi/lo inside blob2s; DVE reconstructs f32
        d_bias = dve([("din3", din_b2)], lambda: nc.vector.tensor_tensor(
            biases[:], bias_hi[:], bias_lo[:], op=OP.add))
        a_tsk1 = act([("dve", d_bias)],
                     lambda: act_i(tsk1buf[:], s2tbk1[:], AF.Tanh, bias=s1tk1[:, 0:1]))

        # st strips: DVE pre-adds (per-partition bias via AP scalar); the
        # first strip waits on d_bias through the semaphore - same-engine
        # RAW (biases written by the previous DVE op) is not ordered by the
        # pipeline alone
        d_sb = {}
        for i in range(NCHUNK):
            w = [("dve", d_bias)] if i == 0 else None
            d_sb[i] = dve(w, lambda i=i: nc.vector.tensor_scalar(
                stba[:, i * T:(i + 1) * T], stb[:], s1col[:, i:i + 1], None, op0=OP.add),
                inc=(i in (3, 7)))
        a_sb1 = act([("dve", d_sb[3])],
                    lambda: act_i(stbv[:, 0:4 * T], stba[:, 0:4 * T], AF.Tanh))
        # packed nv tanh: one [128, 512] pass per column half
        a_Ra = act([("pe", ga)], lambda: act_i(Rp[:, 0:512], zps[0][:, 0:512], AF.Tanh, scale=3.0))
        a_Rb = act([("pe", gb)], lambda: act_i(Rp[:, 512:1024], zps[0][:, 512:1024], AF.Tanh, scale=3.0))
        # second st panel in halves: the first rides the zstep0 window
        a_sb2a = act([("dve", d_sb[7])],
                     lambda: act_i(stbv[:, 4 * T:6 * T], stba[:, 4 * T:6 * T], AF.Tanh))

        # LtBuf = [-nv2; nv1] at the core's own rows (cols 0:1024 of Rp's
        # group 0), built in column pieces as each prep ACT completes; the
        # first 128 cols (chunk 0, j=0 rows) come first so zstep0 starts
        # as soon as possible
        def lt_piece(rows0, cs, w):
            dve(w, lambda: nc.vector.tensor_scalar_mul(
                LtBuf[rows0:rows0 + 32, cs], Rp[32:64, cs], -1.0), inc=False)
            return dve(None, lambda: nc.vector.tensor_copy(
                LtBuf[rows0 + 32:rows0 + 64, cs], Rp[0:32, cs]))

        d_Lt0n = lt_piece(0, slice(0, 128), [("act", a_Ra)])
        d_Lt0w = lt_piece(0, slice(128, 512), None)
        d_Lt0d = lt_piece(64, slice(0, 512), None)
        d_Lt1 = lt_piece(0, slice(512, 1024), [("act", a_Rb)])
        d_Lt1d = lt_piece(64, slice(512, 1024), None)

        # ts k=1 packed: relu + store (dout4); DRAM side rearranged to match
        d_tsk1 = dve([("act", a_tsk1)], lambda: nc.vector.tensor_scalar(
            tsk1buf[:], tsk1buf[:], 0.0, None, op0=OP.max))
        op("sync", [("dve", d_tsk1)],
           lambda: nc.sync.dma_start(
               out=out[NS + 128:NS + TS, 0:N].rearrange("t (b c) -> t b c", b=8),
               in_=tsk1buf[:]),
           "dout4", delta=16)

        # ---------- zsteps (rotation: step s -> zps[(s+2) % 3]) ----------
        # j = s % 2: rhs partition group / PE weight-tile row = 64*j
        zact = []
        pez = []
        z_extra = {0: [("dve", d_Lt0n)], 1: [("dve", d_Lt0d)],
                   2: [("dve", d_Lt0w)],
                   8: [("dve", d_Lt1)], 9: [("dve", d_Lt1d)]}

        # 4-deep rotation from s=6: qtt joins the cycle once the att pass
        # has drained it, so zstep s only waits on zact[s-4] - PE finishes
        # each slot ~1.1us before ACT needs it instead of ~120ns after
        SLOT6 = {6: None, 7: 2, 8: 0, 9: 1}   # None -> qtt

        def zslot(s):
            k = s if s < 6 else 6 + (s - 6) % 4
            if k >= 6:
                k2 = SLOT6[k]
                return qtt if k2 is None else zps[k2]
            return zps[(k + 2) % 3]

        def zstep(s, rs, j):
            waits = list(z_extra.get(s, []))
            if s == 6:
                waits.append(("act", att[0]))
            elif s >= 7:
                waits.append(("act", zact[s - 4]))
            elif s >= 3:
                waits.append(("act", zact[s - 3]))
            p0 = 64 * j
            lhs = LtBuf[p0:p0 + 64, rs]
            w2 = [("act", a_Rb)] if s == 0 else None
            pe(waits, lambda: mm(zslot(s)[:, 0:512], lhs, Rp[p0:p0 + 64, 0:512],
                                 start=True, stop=True))
            g = pe(w2, lambda: mm(zslot(s)[:, 512:1024], lhs,
                                  Rp[p0:p0 + 64, 512:1024], start=True, stop=True), "pe")
            pez.append(g)

        def gtt_mm(pdst, t0, tn, waits):
            if GTT_PASSES == 1:
                return pe(waits, lambda: mm(pdst, t_in["tmrT_hi"][:, t0:t0 + tn], t_in["tmT_hi"][:],
                                            start=True, stop=True), "pe")
            pe(waits, lambda: mm(pdst, t_in["tmrT_hi"][:, t0:t0 + tn], t_in["tmT_hi"][:],
                                 start=True, stop=False))
            if GTT_PASSES == 2:
                return pe(None, lambda: mm(pdst, t_in["tmrT_hi"][:, t0:t0 + tn], t_in["tmT_lo"][:],
                                           start=False, stop=True), "pe")
            pe(None, lambda: mm(pdst, t_in["tmrT_hi"][:, t0:t0 + tn], t_in["tmT_lo"][:],
                                start=False, stop=False))
            return pe(None, lambda: mm(pdst, t_in["tmrT_lo"][:, t0:t0 + tn], t_in["tmT_hi"][:],
                                       start=False, stop=True), "pe")

        outdma = []
        gtts = []
        att = []

        s = 0
        for i in range(NCHUNK):
            rs = slice(i * 128, (i + 1) * 128)
            ob = outbufs[i % 4]
            last = i == NCHUNK - 1
            for j in range(2):
                zstep(s, rs, j)
                if s == 2:
                    gtts.append(gtt_mm(qtt[0:128, 0:T], 0, 128, [("din5", din_tm)]))
                    gtts.append(gtt_mm(qtt[0:TS - 128, 512:512 + T], 128, TS - 128, None))
                ow = [(f"dout{i % 4}", outdma[i - 4])] if (j == 0 and i >= 4) else []
                if last and j == 1:
                    # split the final zact so relu+store overlap its 2nd half
                    zact.append(None)
                    za = act([("pe", pez[s])],
                             lambda ob=ob, s=s: act_i(ob[:, 1024:1536],
                                                      zslot(s)[:, 0:512],
                                                      AF.Tanh, scale=SS_C))
                    zb = act(None,
                             lambda ob=ob, s=s: act_i(ob[:, 1536:2048],
                                                      zslot(s)[:, 512:1024],
                                                      AF.Tanh, scale=SS_C))
                else:
                    zact.append(act([("pe", pez[s])] + ow,
                                    lambda ob=ob, j=j, s=s: act_i(ob[:, j * 1024:(j + 1) * 1024],
                                                                  zslot(s)[:], AF.Tanh,
                                                                  scale=SS_C)))
                s += 1
                # temporal k=0 ts region rides the first zact slack window
                if s == 2:
                    a_ts = act([("din6", din_sF)],
                               lambda: act_i(tob0[:, 0:N], s2tbF[:], AF.Tanh,
                                             bias=s1tcol[:, 0:1]), inc=False)
            if last:
                # tail: ss pieces relu+store independently (st strip is part
                # of the stbv panel, stored mid-pipeline)
                d_a = dve([("act", zact[2 * i])], lambda ob=ob: nc.vector.tensor_scalar(
                    ob[:, 0:1024], ob[:, 0:1024], 0.0, SS_S, op0=OP.max, op1=OP.mult))
                op("sync", [("dve", d_a)],
                   lambda ob=ob, rs=rs: nc.sync.dma_start(out=out[rs, 0:1024],
                                                          in_=ob[:, 0:1024]),
                   f"dout{i % 4}", delta=16)
                d_b1 = dve([("act", za)], lambda ob=ob: nc.vector.tensor_scalar(
                    ob[:, 1024:1536], ob[:, 1024:1536], 0.0, SS_S, op0=OP.max, op1=OP.mult))
                op("sync", [("dve", d_b1)],
                   lambda ob=ob, rs=rs: nc.sync.dma_start(out=out[rs, 1024:1536],
                                                          in_=ob[:, 1024:1536]),
                   f"dout{i % 4}", delta=16)
                d_b2 = dve([("act", zb)], lambda ob=ob: nc.vector.tensor_scalar(
                    ob[:, 1536:2048], ob[:, 1536:2048], 0.0, SS_S, op0=OP.max, op1=OP.mult))
                op("sync", [("dve", d_b2)],
                   lambda ob=ob, rs=rs: nc.sync.dma_start(out=out[rs, 1536:2048],
                                                          in_=ob[:, 1536:2048]),
                   f"dout{i % 4}", delta=16)
            else:
                d_ss = dve([("act", zact[2 * i + 1])], lambda ob=ob: nc.vector.tensor_scalar(
                    ob[:], ob[:], 0.0, SS_S, op0=OP.max, op1=OP.mult))
                outdma.append(op("sync", [("dve", d_ss)],
                                 lambda ob=ob, rs=rs: nc.sync.dma_start(out=out[rs, 0:N], in_=ob[:]),
                                 f"dout{i % 4}", delta=16))
            if i == 0:
                a_sb2b = act([("dve", d_sb[7])],
                             lambda: act_i(stbv[:, 6 * T:8 * T], stba[:, 6 * T:8 * T], AF.Tanh))
            elif i == 1:
                # st panel: relu once, store all 8 strips with one DMA
                d_sv = dve([("act", a_sb2b)], lambda: nc.vector.tensor_scalar(
                    stbv[:], stbv[:], 0.0, None, op0=OP.max))
                op("sync", [("dve", d_sv)],
                   lambda: nc.sync.dma_start(
                       out=out[0:NS, N:NT].rearrange("(i p) t -> p i t", p=128),
                       in_=stbv[:].rearrange("p (i t) -> p i t", t=T)),
                   "dout4", delta=16)
            # interleave temporal ACT work into the stream
            if i == 1:
                # both tt row groups tanh'd in one pass via a strided PSUM
                # AP; emitted here so the s=6 qtt-reuse gate clears before
                # the PE reaches it
                att.append(act([("pe", gtts[1])], lambda: act_i(
                    tttbufM[:], qtt[:, 0:1024].rearrange("p (b c) -> p b c", b=2)[:, :, 0:T],
                    AF.Tanh)))
            elif i == 4:
                # temporal k=0 rows: mask tt, relu, store whole [128, 2336]
                dmm = dve([("act", att[0]), ("din4", din_masks)], lambda: nc.vector.tensor_tensor(
                    tob0[:, N:NT], tttbufM[:, 0:T], mask0[:], op=OP.mult))
                dr = dve([("dve", dmm)], lambda: nc.vector.tensor_scalar(
                    tob0[:], tob0[:], 0.0, None, op0=OP.max))
                op("sync", [("dve", dr)],
                   lambda: nc.sync.dma_start(out=out[NS:NS + 128, :], in_=tob0[:]),
                   "dout4", delta=16)
                # temporal k=1 tt region [16, 288]
                dm1 = dve(None, lambda: nc.vector.tensor_tensor(
                    ttk1buf[:], tttbufM[0:TS - 128, T:2 * T], mask1[:], op=OP.mult))
                dr1 = dve([("dve", dm1)], lambda: nc.vector.tensor_scalar(
                    ttk1buf[:], ttk1buf[:], 0.0, None, op0=OP.max))
                op("sync", [("dve", dr1)],
                   lambda: nc.sync.dma_start(out=out[NS + 128:NS + TS, N:NT], in_=ttk1buf[:]),
                   "dout4", delta=16)

        # ---------- emit (waits embedded into the consuming instruction) ---
        with nc.Block() as block:
            def make_body(engine_name):
                ops = plan[engine_name]

                # waits can be embedded only on single-instruction ops: a
                # matmul emits LDWEIGHTS before the MATMUL that would carry
                # the wait (weights read unguarded), and DMA triggers may
                # start descriptor generation early - both need standalone
                # event-sem waits ahead of the op
                embed_ok = engine_name in ("scalar", "vector", "tensor", "sync")

                def body(eng):
                    satisfied = {}
                    for waits, fn, inc in ops:
                        pend = []
                        for sem_name, val in waits:
                            if val is not None and satisfied.get(sem_name, -1) < val:
                                pend.append((sem_name, val))
                                satisfied[sem_name] = val
                        tail = pend[1:] if embed_ok else pend
                        for sem_name, val in tail:
                            eng.wait_ge(SEM[sem_name], val)
                        ins = fn()
                        if embed_ok and pend:
                            ins.wait_op(SEM[pend[0][0]], pend[0][1], "sem-ge")
                        if inc is None:
                            continue
                        if inc.startswith("din") or inc.startswith("dout"):
                            ins.then_inc(SEM[inc], 16)
                        else:
                            ins.then_inc(SEM[inc], 1)
                return body

            block.sync(make_body("sync"))
            block.tensor(make_body("tensor"))
            block.scalar(make_body("scalar"))
            block.vector(make_body("vector"))

    return nc


def _hilo(a):
    hi = a.astype(np.float16)
    lo = (a - hi.astype(np.float32)).astype(np.float16)
    return hi, lo


def build_in_maps(spatial_nodes, temporal_nodes, W_ss1, W_ss2, w_st, b_st, w_ts, b_ts):
    f = np.float32
    h16 = np.float16
    W12T = np.concatenate([W_ss1.T, W_ss2.T], axis=1).astype(f)
    W_hi, W_lo = _hilo(W12T)
    in_maps = []
    for c in range(N_CORES):
        b, hh = divmod(c, 2)
        tmask = (np.arange(T)[None, :] >= (hh * TS + np.arange(TS))[:, None]).astype(h16)
        # rotate spatial columns so this core's row-half sits at cols 0:NS
        spT = np.ascontiguousarray(np.roll(spatial_nodes[b].T, -hh * NS, axis=1), dtype=f)
        tmT = np.ascontiguousarray(temporal_nodes[b].T, dtype=f)
        sp_hi, sp_lo = _hilo(spT)
        tm_hi, tm_lo = _hilo(tmT)
        parts1 = {"spT_hi": sp_hi, "spT_lo": sp_lo, "W12T_hi": W_hi, "W12T_lo": W_lo}
        blob1 = np.empty((D, B1_W), h16)
        for nm, c0, c1 in B1_SLICES:
            blob1[:, c0:c1] = parts1[nm]
        partsT = {
            "tmT_hi": tm_hi, "tmT_lo": tm_lo,
            "tmrT_hi": tm_hi[:, hh * TS:(hh + 1) * TS],
            "tmrT_lo": tm_lo[:, hh * TS:(hh + 1) * TS],
        }
        tmblob = np.empty((D, TM_W), h16)
        for nm, c0, c1 in TM_SLICES:
            tmblob[:, c0:c1] = partsT[nm]
        # host-side small linear transforms (same class as transpose/hi-lo prep)
        s1 = spT[:, 0:NS].T @ w_st[:D].astype(f)             # [NS]
        s2 = (temporal_nodes[b] @ w_st[D:].astype(f)) + f(b_st)   # [T]
        s1t = temporal_nodes[b, hh * TS:(hh + 1) * TS] @ w_ts[:D].astype(f)  # [TS]
        s2t = spT.T @ w_ts[D:].astype(f) + f(b_ts)           # [N] rotated order
        # k=1 packed: row t*8+blk holds s2t[blk*256 : blk*256+256]
        s2tbk1 = np.ascontiguousarray(s2t.astype(h16).reshape(8, 256)[
            np.tile(np.arange(8), 16), :])
        s2tbF = np.broadcast_to(s2t.astype(h16).ravel(), (128, N)).copy()
        biases = np.zeros((128, NBIAS), f)
        biases[:, 0:NCHUNK] = s1.reshape(NCHUNK, 128).T
        biases[0:128, NCHUNK] = s1t[0:128]
        biases[0:TS - 128, NCHUNK + 1] = s1t[128:TS]
        biases[:, NCHUNK + 2] = np.repeat(s1t[128:TS], 8)
        b_hi, b_lo = _hilo(biases)
        blob2s = np.empty((128, B2S_W), h16)
        blob2s[:, 0:T] = s2.astype(h16)[None, :]
        blob2s[:, T:T + 256] = s2tbk1
        blob2s[:, T + 256:T + 256 + NBIAS] = b_hi
        blob2s[:, T + 256 + NBIAS:] = b_lo
        in_maps.append({
            "blob1": blob1,
            "blob2s": blob2s,
            "s2tbF": s2tbF,
            "tmblob": tmblob,
            "ttmask": tmask,
        })
    return in_maps


def assemble(results):
    out = np.empty((B, NT, NT), np.float32)
    for c in range(N_CORES):
        b, h = divmod(c, 2)
        r = results[c]["out"].astype(np.float32)
        # un-rotate spatial columns (host rotated by -h*NS)
        sp_cols = np.roll(r[:, 0:N], h * NS, axis=1)
        out[b, h * NS:(h + 1) * NS, 0:N] = sp_cols[0:NS]
        out[b, h * NS:(h + 1) * NS, N:NT] = r[0:NS, N:NT]
        out[b, N + h * TS: N + (h + 1) * TS, 0:N] = sp_cols[NS:ROWS]
        out[b, N + h * TS: N + (h + 1) * TS, N:NT] = r[NS:ROWS, N:NT]
    return out


_NC = None


def kernel(**inputs):
    global _NC
    if _NC is None:
        _NC = build_program()
    in_maps = build_in_maps(**inputs)
    res = run_bass_kernel_spmd(_NC, in_maps, list(range(N_CORES)))
    return assemble(res.results)


# revision 42
# speedup vs baseline: 1.0074x; 1.0074x over previous
"""Trainium2 Bass kernel for the MLPSim adjacency-constructor problem.

Full shapes: spatial [4, 2048, 32], temporal [4, 288, 32], output
adj [4, 2336, 2336] f32 where adj = tanh(relu(blocks)):
  ss = tanh(m - m^T), m = nv1 @ nv2^T, nv_i = tanh(3*x@W_i^T)
  st = s1[n] + s2[t] + b_st ;  ts = s1t[t] + s2t[n] + b_ts
  tt = triu(temporal @ temporal^T)

Sharding: 8 cores = (batch b = c//2) x (row-half h = c%2); each core emits
1024 spatial + 144 temporal rows ([1168, 2336]) of one batch. Spatial
COLUMNS are rotated by -h*1024 on the host so each core's row-half sits at
columns 0:1024 (assembly un-rotates).

Device algebra (ACT-bound design, fp16 datapath):
  ss: tanh(relu(tanh(d))) ~= S*relu(tanh(C*d)), minimax S=0.7552623
      C=1.2825139 (maxerr 6.3e-3) -> ONE ACT tanh pass from PSUM + ONE
      DVE dual-op tensor_scalar (max 0, mult S) at 4x fp16 mode.
  Prep z = x@W packs BOTH spatial column-halves into one [128,1024] PSUM
  tile (cols 0:1024 in partitions 0:64, cols 1024:2048 in 64:128); the
  two partition groups use PE weight tiles (0,*) and (0,64) and their
  matmuls overlap on the array, and the nv tanh is 2 ACT passes at half
  the columns. Rp keeps the packed layout; zstep j reads rhs from
  partitions 64j:64j+64 with the weight tile at row 64j (lhsT always
  LtBuf[0:64] = [-nv2; nv1], loaded via explicit tile_position). d
  accumulates in K=64 fp16 matmuls; the PSUM rotation is 3-deep for
  s<6 and 4-deep after (qtt joins the cycle once the att pass drains
  it, emitted one chunk early so the s=6 gate clears in time) - with
  zstep s waiting zact[s-4] the PE fills each slot ~1.1us before ACT
  needs it and the zact phase runs gap-free; do NOT merge adjacent-slot
  zact pairs into one [128,2048] pass (holding two slots starves the PE
  and costs ~1.1us/chunk). prep/tt matmuls use fp16 hi/lo splitting
  (accumulating
  passes at 1 cyc/col), all hi passes first so the late-landing lo blob
  overlaps them. Broadcast/linear tiles (stb, s2tbF, biases as fp16
  hi/lo pairs) are host inputs; no PE broadcasts. Input DMA triggers
  issue from the Sync hwdge queue critical-consumer first, plus one on
  the idle Scalar queue (gpsimd software-DGE triggers measured ~3us
  issue latency - do not use). The st block is built by DVE bias
  pre-adds into one [128, 8*288] strip, tanh'd in three ACT passes,
  relu'd once, and stored with a single rearranged DMA. Temporal ts
  rows are one fused [128,2048] ACT pass in zact slack; both tt row
  groups share one ACT pass via a strided 2-bank PSUM AP. Semaphore
  waits are embedded in the consuming instruction (separate
  EVENT_SEMAPHORE ops cost ~100ns each on the queue). The tail splits
  the last zact into 512-col pieces so relu/store overlap it. All
  stores fp16; host assembles and upcasts.
  Runtime pitfalls baked in: fp32 matmuls run as 2 half-speed passes
  (fp32r is verifier-trapped; fp16 hi/lo instead); 1-partition matmuls
  need their own PSUM bank; same-engine RAW needs a semaphore even
  between adjacent queue entries (biases reconstruct vs first consumer
  raced without one); a DMA semaphore with multiple writers only
  supports all-or-nothing waits; fmap and weights must share the SB
  base partition (hence the LtBuf base-64 duplicate); tanh-degree-11
  polynomials on DVE lose to fp16 intermediate rounding (2.8e-2) - not
  a viable ACT offload.
"""

import numpy as np
from contextlib import ExitStack

import concourse.bass as bass
from concourse import mybir
from concourse.bass_utils import run_bass_kernel_spmd

AF = mybir.ActivationFunctionType
OP = mybir.AluOpType
F32 = mybir.dt.float32
F16 = mybir.dt.float16

B, N, T, D = 4, 2048, 288, 32
NS = N // 2          # 1024 spatial rows per core
TS = T // 2          # 144 temporal rows per core
NT = N + T           # 2336
ROWS = NS + TS       # 1168
N_CORES = 8
NCHUNK = NS // 128   # 8 spatial row-chunks

SS_S = 0.7552623    # tanh(relu(tanh(d))) ~= SS_S * relu(tanh(SS_C*d))
SS_C = 1.2825139

PREP_PASSES = 3      # hi@Whi, hi@Wlo, lo@Whi
GTT_PASSES = 3       # hi.hi, hi.lo, lo.hi

# blob1 [D, B1_W] fp16: prep inputs (W transposed pair, spatial hi/lo)
_B1_DEFS = (("W12T_hi", 2 * D), ("W12T_lo", 2 * D), ("spT_hi", N), ("spT_lo", N))
B1_SLICES = []
_c = 0
for _nm, _w in _B1_DEFS:
    B1_SLICES.append((_nm, _c, _c + _w))
    _c += _w
B1_W = _c

# tmblob [D, TM_W] fp16: temporal hi/lo for the tt matmuls
_TM_DEFS = (("tmT_hi", T), ("tmT_lo", T), ("tmrT_hi", TS), ("tmrT_lo", TS))
TM_SLICES = []
_c = 0
for _nm, _w in _TM_DEFS:
    TM_SLICES.append((_nm, _c, _c + _w))
    _c += _w
TM_W = _c

# blob2s [128, B2S_W] fp16: small early broadcast tiles
#   stb[p, t] = s2[t] + b_st, s2tbk1[t*8+b, c] = s2t[b*256+c],
#   biases packed as fp16 hi/lo pairs (DVE reconstructs f32 on device)
NBIAS = NCHUNK + 3
B2S_W = T + 256 + 2 * NBIAS


def build_program():
    nc = bass.Bass()
    inp = {}

    def di(name, shape, dt=F16):
        inp[name] = nc.declare_dram_parameter(name, list(shape), dt, isOutput=False)

    di("blob1", (D, B1_W))
    di("blob2s", (128, B2S_W))
    di("s2tbF", (128, N))          # host bcast: s2t + b_ts (rotated col order)
    di("tmblob", (D, TM_W))
    di("ttmask", (TS, T))
    out = nc.declare_dram_parameter("out", [ROWS, NT], F16, isOutput=True)

    ctx = ExitStack()
    _uid = [0]

    def sbuf(shape, dt=F16):
        _uid[0] += 1
        return ctx.enter_context(nc.sbuf_tensor(f"sb{_uid[0]}", shape, dt))

    def psum(shape):
        _uid[0] += 1
        return ctx.enter_context(nc.psum_tensor(f"ps{_uid[0]}", shape, F32))

    with ctx:
        blob1 = sbuf([D, B1_W])
        t_in = {nm: blob1[:, c0:c1] for nm, c0, c1 in B1_SLICES}
        tmblob = sbuf([D, TM_W])
        for nm, c0, c1 in TM_SLICES:
            t_in[nm] = tmblob[:, c0:c1]
        blob2s = sbuf([128, B2S_W])
        stb = blob2s[:, 0:T]
        s2tbk1 = blob2s[:, T:T + 256]
        bias_hi = blob2s[:, T + 256:T + 256 + NBIAS]
        bias_lo = blob2s[:, T + 256 + NBIAS:T + 256 + 2 * NBIAS]
        s2tbF = sbuf([128, N])
        dummy = sbuf([1, 16])
        biases = sbuf([128, NBIAS], F32)
        s1col = biases[:, 0:NCHUNK]
        s1tcol = biases[:, NCHUNK:NCHUNK + 2]
        s1tk1 = biases[:, NCHUNK + 2:NCHUNK + 3]
        mask0 = sbuf([128, T])
        mask1 = sbuf([TS - 128, T])
        # packed nv: partitions 0:64 = [nv1; nv2] for spatial cols 0:1024,
        # partitions 64:128 = same for cols 1024:2048
        Rp = sbuf([128, 1024])
        # lhsT: rows 0:32 = -nv2, rows 32:64 = nv1 (at the chunk's rows);
        # rows 64:128 duplicate rows 0:64 (fmap and weights must share the
        # SB base partition, so the j=1 weight tile needs a base-64 copy)
        LtBuf = sbuf([128, NS])
        stba = sbuf([128, NCHUNK * T])       # st strips: stb + s1col[i]
        stbv = sbuf([128, NCHUNK * T])       # tanh(st strips)
        outbufs = [sbuf([128, N]) for _ in range(4)]
        tob0 = sbuf([128, NT])               # temporal k=0 rows
        tsk1buf = sbuf([128, 256])           # temporal k=1 ts region, packed
        ttk1buf = sbuf([TS - 128, T])        # temporal k=1 tt region
        tttbufM = sbuf([128, 2 * T])         # tt tanh: k=0 | k=1 halves

        zps = [psum([128, 1024]) for _ in range(3)]   # 6 banks
        # gtt: k=0 rows in cols 0:T (bank 6), k=1 rows in 512:512+T (bank 7)
        qtt = psum([128, 1024])

        dmain = ctx.enter_context(nc.semaphore("dmain"))
        dmain2 = ctx.enter_context(nc.semaphore("dmain2"))
        dmain3 = ctx.enter_context(nc.semaphore("dmain3"))
        dmain4 = ctx.enter_context(nc.semaphore("dmain4"))
        dmain5 = ctx.enter_context(nc.semaphore("dmain5"))
        dmain6 = ctx.enter_context(nc.semaphore("dmain6"))
        pe_s = ctx.enter_context(nc.semaphore("pe_s"))
        act_s = ctx.enter_context(nc.semaphore("act_s"))
        dve_s = ctx.enter_context(nc.semaphore("dve_s"))
        douts = [ctx.enter_context(nc.semaphore(f"dout{k}")) for k in range(5)]
        SEM = {"pe": pe_s, "act": act_s, "dve": dve_s, "din": dmain,
               "din2": dmain2, "din3": dmain3, "din4": dmain4, "din5": dmain5,
               "din6": dmain6,
               "dout0": douts[0], "dout1": douts[1], "dout2": douts[2],
               "dout3": douts[3], "dout4": douts[4]}

        plan = {"sync": [], "tensor": [], "scalar": [], "vector": [], "gpsimd": []}
        cnt = {"pe": 0, "act": 0, "dve": 0, "din": 0, "din2": 0, "din3": 0,
               "din4": 0, "din5": 0, "din6": 0,
               "dout0": 0, "dout1": 0, "dout2": 0, "dout3": 0, "dout4": 0}

        def op(engine, waits, fn, inc=None, delta=None):
            plan[engine].append((waits or [], fn, inc))
            if inc:
                if delta is None:
                    delta = 16 if inc.startswith("d") and inc != "dve" else 1
                cnt[inc] += delta
                return cnt[inc]
            return None

        # ---------- input loads: sync hwdge queue, priority order ----------
        # blob1 splits at the hi/lo boundary: the lo passes run last on PE,
        # so prep starts ~1.4us earlier on just W + spT_hi
        B1H = 2 * (2 * D) + N
        op("sync", None, lambda: nc.sync.dma_start(out=blob1[:, 0:B1H], in_=inp["blob1"][:, 0:B1H]), "din", delta=16)
        din_b1 = cnt["din"]
        # blob2s rides the Scalar hwdge queue (idle until the table load),
        # landing ~2us earlier than as sync trigger #2
        op("scalar", None, lambda: nc.scalar.dma_start(out=blob2s[:], in_=inp["blob2s"][:]), "din3", delta=16)
        din_b2 = cnt["din3"]
        op("sync", None, lambda: nc.sync.dma_start(out=blob1[:, B1H:B1_W], in_=inp["blob1"][:, B1H:B1_W]), "din2", delta=16)
        din_b1lo = cnt["din2"]
        op("sync", None, lambda: nc.sync.dma_start(out=s2tbF[:], in_=inp["s2tbF"][:]), "din6", delta=16)
        din_sF = cnt["din6"]
        op("sync", None, lambda: nc.sync.dma_start(out=tmblob[:], in_=inp["tmblob"][:]), "din5", delta=16)
        din_tm = cnt["din5"]
        op("sync", None, lambda: nc.sync.dma_start(out=mask0[:], in_=inp["ttmask"][0:128, :]), "din4", delta=16)
        op("sync", None, lambda: nc.sync.dma_start(out=mask1[:], in_=inp["ttmask"][128:TS, :]), "din4", delta=16)
        din_masks = cnt["din4"]

        Whi, Wlo = t_in["W12T_hi"], t_in["W12T_lo"]
        mm = nc.tensor.matmul
        act_i = nc.scalar.activation

        def pe(waits, fn, inc=None):
            return op("tensor", waits, fn, inc)

        def act(waits, fn, inc=True):
            return op("scalar", waits, fn, "act" if inc else None)

        def dve(waits, fn, inc=True):
            return op("vector", waits, fn, "dve" if inc else None)

        # ---------- nv prep: z = x@W, hi/lo accumulate, 128-part packed ----
        # partition group g covers spatial cols [g*1024, (g+1)*1024) at psum
        # cols = local col; both groups' matmuls overlap on the PE array.
        # all hi passes (blob1a) first, lo passes (blob1b) last: the late
        # blob1b landing then overlaps the hi matmuls instead of stalling
        # each region; adjacent different-tile matmuls overlap on the array
        def prep_region(cc, g):
            sc = g * 1024 + cc
            return zps[0][64 * g:64 * (g + 1), cc:cc + 512], sc

        first = [True]
        for hi_t in (Whi, Wlo):
            for cc in (0, 512):
                for g in range(2):
                    dst, sc = prep_region(cc, g)
                    w = [("din", din_b1)] if first[0] else None
                    first[0] = False
                    pe(w, lambda dst=dst, sc=sc, hi_t=hi_t: mm(
                        dst, hi_t[:], t_in["spT_hi"][:, sc:sc + 512],
                        start=hi_t is Whi, stop=False))
        ga = gb = None
        for cc in (0, 512):
            for g in range(2):
                dst, sc = prep_region(cc, g)
                w = [("din2", din_b1lo)] if (cc == 0 and g == 0) else None
                gend = pe(w, lambda dst=dst, sc=sc: mm(
                    dst, Whi[:], t_in["spT_lo"][:, sc:sc + 512],
                    start=False, stop=True), "pe")
            if cc == 0:
                ga = gend
            else:
                gb = gend

        # ---------- ACT stream: warm, early fill, prep tanh, zacts ---------
        d_dum = dve(None, lambda: nc.vector.memset(dummy[:], 0.25))
        act([("dve", d_dum)], lambda: act_i(dummy[0:1, 8:16], dummy[0:1, 0:8], AF.Tanh),
            inc=False)  # warms the Tanh table during the input-DMA wait

        # biases arrive as fp16 hi/lo inside blob2s; DVE reconstructs f32
        d_bias = dve([("din3", din_b2)], lambda: nc.vector.tensor_tensor(
            biases[:], bias_hi[:], bias_lo[:], op=OP.add))
        a_tsk1 = act([("dve", d_bias)],
                     lambda: act_i(tsk1buf[:], s2tbk1[:], AF.Tanh, bias=s1tk1[:, 0:1]))

        # st strips: DVE pre-adds (per-partition bias via AP scalar); the
        # first strip waits on d_bias through the semaphore - same-engine
        # RAW (biases written by the previous DVE op) is not ordered by the
        # pipeline alone
        d_sb = {}
        for i in range(NCHUNK):
            w = [("dve", d_bias)] if i == 0 else None
            d_sb[i] = dve(w, lambda i=i: nc.vector.tensor_scalar(
                stba[:, i * T:(i + 1) * T], stb[:], s1col[:, i:i + 1], None, op0=OP.add),
                inc=(i in (3, 7)))
        a_sb1 = act([("dve", d_sb[3])],
                    lambda: act_i(stbv[:, 0:4 * T], stba[:, 0:4 * T], AF.Tanh))
        # packed nv tanh: one [128, 512] pass per column half
        a_Ra = act([("pe", ga)], lambda: act_i(Rp[:, 0:512], zps[0][:, 0:512], AF.Tanh, scale=3.0))
        a_Rb = act([("pe", gb)], lambda: act_i(Rp[:, 512:1024], zps[0][:, 512:1024], AF.Tanh, scale=3.0))
        # second st panel in halves: the first rides the zstep0 window
        a_sb2a = act([("dve", d_sb[7])],
                     lambda: act_i(stbv[:, 4 * T:6 * T], stba[:, 4 * T:6 * T], AF.Tanh))

        # LtBuf = [-nv2; nv1] at the core's own rows (cols 0:1024 of Rp's
        # group 0), built in column pieces as each prep ACT completes; the
        # first 128 cols (chunk 0, j=0 rows) come first so zstep0 starts
        # as soon as possible
        def lt_piece(rows0, cs, w):
            dve(w, lambda: nc.vector.tensor_scalar_mul(
                LtBuf[rows0:rows0 + 32, cs], Rp[32:64, cs], -1.0), inc=False)
            return dve(None, lambda: nc.vector.tensor_copy(
                LtBuf[rows0 + 32:rows0 + 64, cs], Rp[0:32, cs]))

        d_Lt0n = lt_piece(0, slice(0, 128), [("act", a_Ra)])
        d_Lt0w = lt_piece(0, slice(128, 512), None)
        d_Lt0d = lt_piece(64, slice(0, 512), None)
        d_Lt1 = lt_piece(0, slice(512, 1024), [("act", a_Rb)])
        d_Lt1d = lt_piece(64, slice(512, 1024), None)

        # ts k=1 packed: relu + store (dout4); DRAM side rearranged to match
        d_tsk1 = dve([("act", a_tsk1)], lambda: nc.vector.tensor_scalar(
            tsk1buf[:], tsk1buf[:], 0.0, None, op0=OP.max))
        op("sync", [("dve", d_tsk1)],
           lambda: nc.sync.dma_start(
               out=out[NS + 128:NS + TS, 0:N].rearrange("t (b c) -> t b c", b=8),
               in_=tsk1buf[:]),
           "dout4", delta=16)

        # ---------- zsteps (rotation: step s -> zps[(s+2) % 3]) ----------
        # j = s % 2: rhs partition group / PE weight-tile row = 64*j
        zact = []
        pez = []
        z_extra = {0: [("dve", d_Lt0n)], 1: [("dve", d_Lt0d)],
                   2: [("dve", d_Lt0w)],
                   8: [("dve", d_Lt1)], 9: [("dve", d_Lt1d)]}

        # 4-deep rotation from s=6: qtt joins the cycle once the att pass
        # has drained it, so zstep s only waits on zact[s-4] - PE finishes
        # each slot ~1.1us before ACT needs it instead of ~120ns after
        SLOT6 = {6: None, 7: 2, 8: 0, 9: 1}   # None -> qtt

        def zslot(s):
            k = s if s < 6 else 6 + (s - 6) % 4
            if k >= 6:
                k2 = SLOT6[k]
                return qtt if k2 is None else zps[k2]
            return zps[(k + 2) % 3]

        def zstep(s, rs, j):
            waits = list(z_extra.get(s, []))
            if s == 6:
                waits.append(("act", att[0]))
            elif s >= 7:
                waits.append(("act", zact[s - 4]))
            elif s >= 3:
                waits.append(("act", zact[s - 3]))
            p0 = 64 * j
            lhs = LtBuf[p0:p0 + 64, rs]
            w2 = [("act", a_Rb)] if s == 0 else None
            pe(waits, lambda: mm(zslot(s)[:, 0:512], lhs, Rp[p0:p0 + 64, 0:512],
                                 start=True, stop=True))
            g = pe(w2, lambda: mm(zslot(s)[:, 512:1024], lhs,
                                  Rp[p0:p0 + 64, 512:1024], start=True, stop=True), "pe")
            pez.append(g)

        def gtt_mm(pdst, t0, tn, waits):
            if GTT_PASSES == 1:
                return pe(waits, lambda: mm(pdst, t_in["tmrT_hi"][:, t0:t0 + tn], t_in["tmT_hi"][:],
                                            start=True, stop=True), "pe")
            pe(waits, lambda: mm(pdst, t_in["tmrT_hi"][:, t0:t0 + tn], t_in["tmT_hi"][:],
                                 start=True, stop=False))
            if GTT_PASSES == 2:
                return pe(None, lambda: mm(pdst, t_in["tmrT_hi"][:, t0:t0 + tn], t_in["tmT_lo"][:],
                                           start=False, stop=True), "pe")
            pe(None, lambda: mm(pdst, t_in["tmrT_hi"][:, t0:t0 + tn], t_in["tmT_lo"][:],
                                start=False, stop=False))
            return pe(None, lambda: mm(pdst, t_in["tmrT_lo"][:, t0:t0 + tn], t_in["tmT_hi"][:],
                                       start=False, stop=True), "pe")

        outdma = []
        gtts = []
        att = []

        s = 0
        for i in range(NCHUNK):
            rs = slice(i * 128, (i + 1) * 128)
            ob = outbufs[i % 4]
            last = i == NCHUNK - 1
            for j in range(2):
                zstep(s, rs, j)
                if s == 2:
                    gtts.append(gtt_mm(qtt[0:128, 0:T], 0, 128, [("din5", din_tm)]))
                    gtts.append(gtt_mm(qtt[0:TS - 128, 512:512 + T], 128, TS - 128, None))
                ow = [(f"dout{i % 4}", outdma[i - 4])] if (j == 0 and i >= 4) else []
                if last and j == 1:
                    # split the final zact so relu+store overlap its 2nd half
                    zact.append(None)
                    za = act([("pe", pez[s])],
                             lambda ob=ob, s=s: act_i(ob[:, 1024:1536],
                                                      zslot(s)[:, 0:512],
                                                      AF.Tanh, scale=SS_C))
                    zb = act(None,
                             lambda ob=ob, s=s: act_i(ob[:, 1536:2048],
                                                      zslot(s)[:, 512:1024],
                                                      AF.Tanh, scale=SS_C))
                else:
                    zact.append(act([("pe", pez[s])] + ow,
                                    lambda ob=ob, j=j, s=s: act_i(ob[:, j * 1024:(j + 1) * 1024],
                                                                  zslot(s)[:], AF.Tanh,
                                                                  scale=SS_C)))
                s += 1
                # temporal k=0 ts region rides the first zact slack window
                if s == 2:
                    a_ts = act([("din6", din_sF)],
                               lambda: act_i(tob0[:, 0:N], s2tbF[:], AF.Tanh,
                                             bias=s1tcol[:, 0:1]), inc=False)
            if last:
                # tail: ss pieces relu+store independently (st strip is part
                # of the stbv panel, stored mid-pipeline)
                d_a = dve([("act", zact[2 * i])], lambda ob=ob: nc.vector.tensor_scalar(
                    ob[:, 0:1024], ob[:, 0:1024], 0.0, SS_S, op0=OP.max, op1=OP.mult))
                op("sync", [("dve", d_a)],
                   lambda ob=ob, rs=rs: nc.sync.dma_start(out=out[rs, 0:1024],
                                                          in_=ob[:, 0:1024]),
                   f"dout{i % 4}", delta=16)
                d_b1 = dve([("act", za)], lambda ob=ob: nc.vector.tensor_scalar(
                    ob[:, 1024:1536], ob[:, 1024:1536], 0.0, SS_S, op0=OP.max, op1=OP.mult))
                op("sync", [("dve", d_b1)],
                   lambda ob=ob, rs=rs: nc.sync.dma_start(out=out[rs, 1024:1536],
                                                          in_=ob[:, 1024:1536]),
                   f"dout{i % 4}", delta=16)
                d_b2 = dve([("act", zb)], lambda ob=ob: nc.vector.tensor_scalar(
                    ob[:, 1536:2048], ob[:, 1536:2048], 0.0, SS_S, op0=OP.max, op1=OP.mult))
                # final store triggers from the now-idle Scalar hwdge queue
                # so it doesn't serialize behind the previous store on Sync
                op("scalar", [("dve", d_b2)],
                   lambda ob=ob, rs=rs: nc.scalar.dma_start(out=out[rs, 1536:2048],
                                                            in_=ob[:, 1536:2048]),
                   f"dout{i % 4}", delta=16)
            else:
                d_ss = dve([("act", zact[2 * i + 1])], lambda ob=ob: nc.vector.tensor_scalar(
                    ob[:], ob[:], 0.0, SS_S, op0=OP.max, op1=OP.mult))
                outdma.append(op("sync", [("dve", d_ss)],
                                 lambda ob=ob, rs=rs: nc.sync.dma_start(out=out[rs, 0:N], in_=ob[:]),
                                 f"dout{i % 4}", delta=16))
            if i == 0:
                a_sb2b = act([("dve", d_sb[7])],
                             lambda: act_i(stbv[:, 6 * T:8 * T], stba[:, 6 * T:8 * T], AF.Tanh))
            elif i == 1:
                # st panel: relu once, store all 8 strips with one DMA
                d_sv = dve([("act", a_sb2b)], lambda: nc.vector.tensor_scalar(
                    stbv[:], stbv[:], 0.0, None, op0=OP.max))
                op("sync", [("dve", d_sv)],
                   lambda: nc.sync.dma_start(
                       out=out[0:NS, N:NT].rearrange("(i p) t -> p i t", p=128),
                       in_=stbv[:].rearrange("p (i t) -> p i t", t=T)),
                   "dout4", delta=16)
            # interleave temporal ACT work into the stream
            if i == 1:
                # both tt row groups tanh'd in one pass via a strided PSUM
                # AP; emitted here so the s=6 qtt-reuse gate clears before
                # the PE reaches it
                att.append(act([("pe", gtts[1])], lambda: act_i(
                    tttbufM[:], qtt[:, 0:1024].rearrange("p (b c) -> p b c", b=2)[:, :, 0:T],
                    AF.Tanh)))
            elif i == 4:
                # temporal k=0 rows: mask tt, relu, store whole [128, 2336]
                dmm = dve([("act", att[0]), ("din4", din_masks)], lambda: nc.vector.tensor_tensor(
                    tob0[:, N:NT], tttbufM[:, 0:T], mask0[:], op=OP.mult))
                dr = dve([("dve", dmm)], lambda: nc.vector.tensor_scalar(
                    tob0[:], tob0[:], 0.0, None, op0=OP.max))
                op("sync", [("dve", dr)],
                   lambda: nc.sync.dma_start(out=out[NS:NS + 128, :], in_=tob0[:]),
                   "dout4", delta=16)
                # temporal k=1 tt region [16, 288]
                dm1 = dve(None, lambda: nc.vector.tensor_tensor(
                    ttk1buf[:], tttbufM[0:TS - 128, T:2 * T], mask1[:], op=OP.mult))
                dr1 = dve([("dve", dm1)], lambda: nc.vector.tensor_scalar(
                    ttk1buf[:], ttk1buf[:], 0.0, None, op0=OP.max))
                op("sync", [("dve", dr1)],
                   lambda: nc.sync.dma_start(out=out[NS + 128:NS + TS, N:NT], in_=ttk1buf[:]),
                   "dout4", delta=16)

        # ---------- emit (waits embedded into the consuming instruction) ---
        with nc.Block() as block:
            def make_body(engine_name):
                ops = plan[engine_name]

                # waits can be embedded only on single-instruction ops: a
                # matmul emits LDWEIGHTS before the MATMUL that would carry
                # the wait (weights read unguarded), and DMA triggers may
                # start descriptor generation early - both need standalone
                # event-sem waits ahead of the op
                embed_ok = engine_name in ("scalar", "vector", "tensor", "sync")

                def body(eng):
                    satisfied = {}
                    for waits, fn, inc in ops:
                        pend = []
                        for sem_name, val in waits:
                            if val is not None and satisfied.get(sem_name, -1) < val:
                                pend.append((sem_name, val))
                                satisfied[sem_name] = val
                        tail = pend[1:] if embed_ok else pend
                        for sem_name, val in tail:
                            eng.wait_ge(SEM[sem_name], val)
                        ins = fn()
                        if embed_ok and pend:
                            ins.wait_op(SEM[pend[0][0]], pend[0][1], "sem-ge")
                        if inc is None:
                            continue
                        if inc.startswith("din") or inc.startswith("dout"):
                            ins.then_inc(SEM[inc], 16)
                        else:
                            ins.then_inc(SEM[inc], 1)
                return body

            block.sync(make_body("sync"))
            block.tensor(make_body("tensor"))
            block.scalar(make_body("scalar"))
            block.vector(make_body("vector"))

    return nc


def _hilo(a):
    hi = a.astype(np.float16)
    lo = (a - hi.astype(np.float32)).astype(np.float16)
    return hi, lo


def build_in_maps(spatial_nodes, temporal_nodes, W_ss1, W_ss2, w_st, b_st, w_ts, b_ts):
    f = np.float32
    h16 = np.float16
    W12T = np.concatenate([W_ss1.T, W_ss2.T], axis=1).astype(f)
    W_hi, W_lo = _hilo(W12T)
    in_maps = []
    for c in range(N_CORES):
        b, hh = divmod(c, 2)
        tmask = (np.arange(T)[None, :] >= (hh * TS + np.arange(TS))[:, None]).astype(h16)
        # rotate spatial columns so this core's row-half sits at cols 0:NS
        spT = np.ascontiguousarray(np.roll(spatial_nodes[b].T, -hh * NS, axis=1), dtype=f)
        tmT = np.ascontiguousarray(temporal_nodes[b].T, dtype=f)
        sp_hi, sp_lo = _hilo(spT)
        tm_hi, tm_lo = _hilo(tmT)
        parts1 = {"spT_hi": sp_hi, "spT_lo": sp_lo, "W12T_hi": W_hi, "W12T_lo": W_lo}
        blob1 = np.empty((D, B1_W), h16)
        for nm, c0, c1 in B1_SLICES:
            blob1[:, c0:c1] = parts1[nm]
        partsT = {
            "tmT_hi": tm_hi, "tmT_lo": tm_lo,
            "tmrT_hi": tm_hi[:, hh * TS:(hh + 1) * TS],
            "tmrT_lo": tm_lo[:, hh * TS:(hh + 1) * TS],
        }
        tmblob = np.empty((D, TM_W), h16)
        for nm, c0, c1 in TM_SLICES:
            tmblob[:, c0:c1] = partsT[nm]
        # host-side small linear transforms (same class as transpose/hi-lo prep)
        s1 = spT[:, 0:NS].T @ w_st[:D].astype(f)             # [NS]
        s2 = (temporal_nodes[b] @ w_st[D:].astype(f)) + f(b_st)   # [T]
        s1t = temporal_nodes[b, hh * TS:(hh + 1) * TS] @ w_ts[:D].astype(f)  # [TS]
        s2t = spT.T @ w_ts[D:].astype(f) + f(b_ts)           # [N] rotated order
        # k=1 packed: row t*8+blk holds s2t[blk*256 : blk*256+256]
        s2tbk1 = np.ascontiguousarray(s2t.astype(h16).reshape(8, 256)[
            np.tile(np.arange(8), 16), :])
        s2tbF = np.broadcast_to(s2t.astype(h16).ravel(), (128, N)).copy()
        biases = np.zeros((128, NBIAS), f)
        biases[:, 0:NCHUNK] = s1.reshape(NCHUNK, 128).T
        biases[0:128, NCHUNK] = s1t[0:128]
        biases[0:TS - 128, NCHUNK + 1] = s1t[128:TS]
        biases[:, NCHUNK + 2] = np.repeat(s1t[128:TS], 8)
        b_hi, b_lo = _hilo(biases)
        blob2s = np.empty((128, B2S_W), h16)
        blob2s[:, 0:T] = s2.astype(h16)[None, :]
        blob2s[:, T:T + 256] = s2tbk1
        blob2s[:, T + 256:T + 256 + NBIAS] = b_hi
        blob2s[:, T + 256 + NBIAS:] = b_lo
        in_maps.append({
            "blob1": blob1,
            "blob2s": blob2s,
            "s2tbF": s2tbF,
            "tmblob": tmblob,
            "ttmask": tmask,
        })
    return in_maps


def assemble(results):
    out = np.empty((B, NT, NT), np.float32)
    for c in range(N_CORES):
        b, h = divmod(c, 2)
        r = results[c]["out"].astype(np.float32)
        # un-rotate spatial columns (host rotated by -h*NS)
        sp_cols = np.roll(r[:, 0:N], h * NS, axis=1)
        out[b, h * NS:(h + 1) * NS, 0:N] = sp_cols[0:NS]
        out[b, h * NS:(h + 1) * NS, N:NT] = r[0:NS, N:NT]
        out[b, N + h * TS: N + (h + 1) * TS, 0:N] = sp_cols[NS:ROWS]
        out[b, N + h * TS: N + (h + 1) * TS, N:NT] = r[NS:ROWS, N:NT]
    return out


_NC = None


def kernel(**inputs):
    global _NC
    if _NC is None:
        _NC = build_program()
    in_maps = build_in_maps(**inputs)
    res = run_bass_kernel_spmd(_NC, in_maps, list(range(N_CORES)))
    return assemble(res.results)


# revision 44
# speedup vs baseline: 1.0147x; 1.0073x over previous
"""Trainium2 Bass kernel for the MLPSim adjacency-constructor problem.

Full shapes: spatial [4, 2048, 32], temporal [4, 288, 32], output
adj [4, 2336, 2336] f32 where adj = tanh(relu(blocks)):
  ss = tanh(m - m^T), m = nv1 @ nv2^T, nv_i = tanh(3*x@W_i^T)
  st = s1[n] + s2[t] + b_st ;  ts = s1t[t] + s2t[n] + b_ts
  tt = triu(temporal @ temporal^T)

Sharding: 8 cores = (batch b = c//2) x (row-half h = c%2); each core emits
1024 spatial + 144 temporal rows ([1168, 2336]) of one batch. Spatial
COLUMNS are rotated by -h*1024 on the host so each core's row-half sits at
columns 0:1024 (assembly un-rotates).

Device algebra (ACT-bound design, fp16 datapath):
  ss: tanh(relu(tanh(d))) ~= S*relu(tanh(C*d)), minimax S=0.7552623
      C=1.2825139 (maxerr 6.3e-3) -> ONE ACT tanh pass from PSUM + ONE
      DVE dual-op tensor_scalar (max 0, mult S) at 4x fp16 mode.
  Prep z = x@W packs BOTH spatial column-halves into one [128,1024] PSUM
  tile (cols 0:1024 in partitions 0:64, cols 1024:2048 in 64:128); the
  two partition groups use PE weight tiles (0,*) and (0,64) and their
  matmuls overlap on the array, and the nv tanh is 2 ACT passes at half
  the columns. Rp keeps the packed layout; zstep j reads rhs from
  partitions 64j:64j+64 with the weight tile at row 64j (lhsT always
  LtBuf[0:64] = [-nv2; nv1], loaded via explicit tile_position). d
  accumulates in K=64 fp16 matmuls; the PSUM rotation is 3-deep for
  s<6 and 4-deep after (qtt joins the cycle once the att pass drains
  it, emitted one chunk early so the s=6 gate clears in time) - with
  zstep s waiting zact[s-4] the PE fills each slot ~1.1us before ACT
  needs it and the zact phase runs gap-free; do NOT merge adjacent-slot
  zact pairs into one [128,2048] pass (holding two slots starves the PE
  and costs ~1.1us/chunk). prep/tt matmuls use fp16 hi/lo splitting
  (accumulating
  passes at 1 cyc/col), all hi passes first so the late-landing lo blob
  overlaps them. Broadcast/linear tiles (stb, s2tbF, biases as fp16
  hi/lo pairs) are host inputs; no PE broadcasts. Input DMA triggers
  issue from the Sync hwdge queue critical-consumer first, plus one on
  the idle Scalar queue (gpsimd software-DGE triggers measured ~3us
  issue latency - do not use). The st block is built by DVE bias
  pre-adds into one [128, 8*288] strip, tanh'd in three ACT passes,
  relu'd once, and stored with a single rearranged DMA. Temporal ts
  rows are one fused [128,2048] ACT pass in zact slack; both tt row
  groups share one ACT pass via a strided 2-bank PSUM AP. Semaphore
  waits are embedded in the consuming instruction (separate
  EVENT_SEMAPHORE ops cost ~100ns each on the queue). The tail splits
  the last zact into 512-col pieces so relu/store overlap it. All
  stores fp16; host assembles and upcasts.
  Runtime pitfalls baked in: fp32 matmuls run as 2 half-speed passes
  (fp32r is verifier-trapped; fp16 hi/lo instead); 1-partition matmuls
  need their own PSUM bank; same-engine RAW needs a semaphore even
  between adjacent queue entries (biases reconstruct vs first consumer
  raced without one); a DMA semaphore with multiple writers only
  supports all-or-nothing waits; fmap and weights must share the SB
  base partition (hence the LtBuf base-64 duplicate); tanh-degree-11
  polynomials on DVE lose to fp16 intermediate rounding (2.8e-2) - not
  a viable ACT offload.
"""

import numpy as np
from contextlib import ExitStack

import concourse.bass as bass
from concourse import mybir
from concourse.bass_utils import run_bass_kernel_spmd

AF = mybir.ActivationFunctionType
OP = mybir.AluOpType
F32 = mybir.dt.float32
F16 = mybir.dt.float16

B, N, T, D = 4, 2048, 288, 32
NS = N // 2          # 1024 spatial rows per core
TS = T // 2          # 144 temporal rows per core
NT = N + T           # 2336
ROWS = NS + TS       # 1168
N_CORES = 8
NCHUNK = NS // 128   # 8 spatial row-chunks

SS_S = 0.7552623    # tanh(relu(tanh(d))) ~= SS_S * relu(tanh(SS_C*d))
SS_C = 1.2825139

PREP_PASSES = 3      # hi@Whi, hi@Wlo, lo@Whi
GTT_PASSES = 3       # hi.hi, hi.lo, lo.hi

# blob1 [D, B1_W] fp16: prep inputs (W transposed pair, spatial hi/lo)
_B1_DEFS = (("W12T_hi", 2 * D), ("W12T_lo", 2 * D), ("spT_hi", N), ("spT_lo", N))
B1_SLICES = []
_c = 0
for _nm, _w in _B1_DEFS:
    B1_SLICES.append((_nm, _c, _c + _w))
    _c += _w
B1_W = _c

# tmblob [D, TM_W] fp16: temporal hi/lo for the tt matmuls
_TM_DEFS = (("tmT_hi", T), ("tmT_lo", T), ("tmrT_hi", TS), ("tmrT_lo", TS))
TM_SLICES = []
_c = 0
for _nm, _w in _TM_DEFS:
    TM_SLICES.append((_nm, _c, _c + _w))
    _c += _w
TM_W = _c

# blob2s [128, B2S_W] fp16: small early broadcast tiles
#   stb[p, t] = s2[t] + b_st, s2tbk1[t*8+b, c] = s2t[b*256+c],
#   biases packed as fp16 hi/lo pairs (DVE reconstructs f32 on device)
NBIAS = NCHUNK + 3
B2S_W = T + 256 + 2 * NBIAS


def build_program():
    nc = bass.Bass()
    inp = {}

    def di(name, shape, dt=F16):
        inp[name] = nc.declare_dram_parameter(name, list(shape), dt, isOutput=False)

    di("blob1", (D, B1_W))
    di("blob2s", (128, B2S_W))
    di("s2tbF", (128, N))          # host bcast: s2t + b_ts (rotated col order)
    di("tmblob", (D, TM_W))
    di("ttmask", (TS, T))
    out = nc.declare_dram_parameter("out", [ROWS, NT], F16, isOutput=True)

    ctx = ExitStack()
    _uid = [0]

    def sbuf(shape, dt=F16):
        _uid[0] += 1
        return ctx.enter_context(nc.sbuf_tensor(f"sb{_uid[0]}", shape, dt))

    def psum(shape):
        _uid[0] += 1
        return ctx.enter_context(nc.psum_tensor(f"ps{_uid[0]}", shape, F32))

    with ctx:
        blob1 = sbuf([D, B1_W])
        t_in = {nm: blob1[:, c0:c1] for nm, c0, c1 in B1_SLICES}
        tmblob = sbuf([D, TM_W])
        for nm, c0, c1 in TM_SLICES:
            t_in[nm] = tmblob[:, c0:c1]
        blob2s = sbuf([128, B2S_W])
        stb = blob2s[:, 0:T]
        s2tbk1 = blob2s[:, T:T + 256]
        bias_hi = blob2s[:, T + 256:T + 256 + NBIAS]
        bias_lo = blob2s[:, T + 256 + NBIAS:T + 256 + 2 * NBIAS]
        s2tbF = sbuf([128, N])
        dummy = sbuf([1, 16])
        biases = sbuf([128, NBIAS], F32)
        s1col = biases[:, 0:NCHUNK]
        s1tcol = biases[:, NCHUNK:NCHUNK + 2]
        s1tk1 = biases[:, NCHUNK + 2:NCHUNK + 3]
        mask0 = sbuf([128, T])
        mask1 = sbuf([TS - 128, T])
        # packed nv: partitions 0:64 = [nv1; nv2] for spatial cols 0:1024,
        # partitions 64:128 = same for cols 1024:2048
        Rp = sbuf([128, 1024])
        # lhsT: rows 0:32 = -nv2, rows 32:64 = nv1 (at the chunk's rows);
        # rows 64:128 duplicate rows 0:64 (fmap and weights must share the
        # SB base partition, so the j=1 weight tile needs a base-64 copy)
        LtBuf = sbuf([128, NS])
        stba = sbuf([128, NCHUNK * T])       # st strips: stb + s1col[i]
        stbv = sbuf([128, NCHUNK * T])       # tanh(st strips)
        outbufs = [sbuf([128, N]) for _ in range(4)]
        tob0 = sbuf([128, NT])               # temporal k=0 rows
        tsk1buf = sbuf([128, 256])           # temporal k=1 ts region, packed
        ttk1buf = sbuf([TS - 128, T])        # temporal k=1 tt region
        tttbufM = sbuf([128, 2 * T])         # tt tanh: k=0 | k=1 halves

        zps = [psum([128, 1024]) for _ in range(3)]   # 6 banks
        # gtt: k=0 rows in cols 0:T (bank 6), k=1 rows in 512:512+T (bank 7)
        qtt = psum([128, 1024])

        dmain = ctx.enter_context(nc.semaphore("dmain"))
        dmain2 = ctx.enter_context(nc.semaphore("dmain2"))
        dmain3 = ctx.enter_context(nc.semaphore("dmain3"))
        dmain4 = ctx.enter_context(nc.semaphore("dmain4"))
        dmain5 = ctx.enter_context(nc.semaphore("dmain5"))
        dmain6 = ctx.enter_context(nc.semaphore("dmain6"))
        dmain7 = ctx.enter_context(nc.semaphore("dmain7"))
        pe_s = ctx.enter_context(nc.semaphore("pe_s"))
        act_s = ctx.enter_context(nc.semaphore("act_s"))
        dve_s = ctx.enter_context(nc.semaphore("dve_s"))
        douts = [ctx.enter_context(nc.semaphore(f"dout{k}")) for k in range(5)]
        SEM = {"pe": pe_s, "act": act_s, "dve": dve_s, "din": dmain,
               "din2": dmain2, "din3": dmain3, "din4": dmain4, "din5": dmain5,
               "din6": dmain6, "din7": dmain7,
               "dout0": douts[0], "dout1": douts[1], "dout2": douts[2],
               "dout3": douts[3], "dout4": douts[4]}

        plan = {"sync": [], "tensor": [], "scalar": [], "vector": [], "gpsimd": []}
        cnt = {"pe": 0, "act": 0, "dve": 0, "din": 0, "din2": 0, "din3": 0,
               "din4": 0, "din5": 0, "din6": 0, "din7": 0,
               "dout0": 0, "dout1": 0, "dout2": 0, "dout3": 0, "dout4": 0}

        def op(engine, waits, fn, inc=None, delta=None):
            plan[engine].append((waits or [], fn, inc))
            if inc:
                if delta is None:
                    delta = 16 if inc.startswith("d") and inc != "dve" else 1
                cnt[inc] += delta
                return cnt[inc]
            return None

        # ---------- input loads: sync hwdge queue, priority order ----------
        # blob1 arrives as 3 progressive DMAs; host orders spatial blocks
        # [A=0:512, C=1024:1536, B=512:1024, D=1536:2048] for hi then lo,
        # so the first DMA (W + A + C) feeds the first matmul pair at once
        B1H1 = 2 * (2 * D) + 1024
        B1H2 = 2 * (2 * D) + N
        op("sync", None, lambda: nc.sync.dma_start(out=blob1[:, 0:B1H1], in_=inp["blob1"][:, 0:B1H1]), "din", delta=16)
        din_b1 = cnt["din"]
        # blob2s rides the Scalar hwdge queue (idle until the table load)
        op("scalar", None, lambda: nc.scalar.dma_start(out=blob2s[:], in_=inp["blob2s"][:]), "din3", delta=16)
        din_b2 = cnt["din3"]
        op("sync", None, lambda: nc.sync.dma_start(out=blob1[:, B1H1:B1H2], in_=inp["blob1"][:, B1H1:B1H2]), "din7", delta=16)
        din_b1h2 = cnt["din7"]
        op("sync", None, lambda: nc.sync.dma_start(out=blob1[:, B1H2:B1_W], in_=inp["blob1"][:, B1H2:B1_W]), "din2", delta=16)
        din_b1lo = cnt["din2"]
        op("sync", None, lambda: nc.sync.dma_start(out=s2tbF[:], in_=inp["s2tbF"][:]), "din6", delta=16)
        din_sF = cnt["din6"]
        op("sync", None, lambda: nc.sync.dma_start(out=tmblob[:], in_=inp["tmblob"][:]), "din5", delta=16)
        din_tm = cnt["din5"]
        op("sync", None, lambda: nc.sync.dma_start(out=mask0[:], in_=inp["ttmask"][0:128, :]), "din4", delta=16)
        op("sync", None, lambda: nc.sync.dma_start(out=mask1[:], in_=inp["ttmask"][128:TS, :]), "din4", delta=16)
        din_masks = cnt["din4"]

        Whi, Wlo = t_in["W12T_hi"], t_in["W12T_lo"]
        mm = nc.tensor.matmul
        act_i = nc.scalar.activation

        def pe(waits, fn, inc=None):
            return op("tensor", waits, fn, inc)

        def act(waits, fn, inc=True):
            return op("scalar", waits, fn, "act" if inc else None)

        def dve(waits, fn, inc=True):
            return op("vector", waits, fn, "dve" if inc else None)

        # ---------- nv prep: z = x@W, hi/lo accumulate, 128-part packed ----
        # partition group g covers spatial cols [g*1024, (g+1)*1024) at psum
        # cols = local col; both groups' matmuls overlap on the PE array.
        # blob1 block layout: hi blocks [A, C, B, D] at B1H0+pos*512, lo
        # blocks in the same order afterwards; PSUM region (g, cc) maps to
        # pos {(0,0):0, (1,0):1, (0,512):2, (1,512):3} so each (A,C)/(B,D)
        # pair overlaps as two PE weight tiles
        B1H0 = 2 * (2 * D)
        B1LO = B1H0 + N
        POS = {(0, 0): 0, (1, 0): 1, (0, 512): 2, (1, 512): 3}

        ga = gb = None
        for pi, (W_, base) in enumerate(((Whi, B1H0), (Wlo, B1H0), (Whi, B1LO))):
            for cc in (0, 512):
                for g in range(2):
                    dst = zps[0][64 * g:64 * (g + 1), cc:cc + 512]
                    c0 = base + POS[(g, cc)] * 512
                    w = None
                    if g == 0 and pi == 0:
                        w = [("din", din_b1)] if cc == 0 else [("din7", din_b1h2)]
                    elif g == 0 and cc == 0 and pi == 2:
                        w = [("din2", din_b1lo)]
                    gend = pe(w, lambda dst=dst, W_=W_, c0=c0, pi=pi: mm(
                        dst, W_[:], blob1[:, c0:c0 + 512],
                        start=pi == 0, stop=pi == 2),
                        "pe" if pi == 2 else None)
                if pi == 2 and cc == 0:
                    ga = gend
                elif pi == 2:
                    gb = gend

        # ---------- ACT stream: warm, early fill, prep tanh, zacts ---------
        d_dum = dve(None, lambda: nc.vector.memset(dummy[:], 0.25))
        act([("dve", d_dum)], lambda: act_i(dummy[0:1, 8:16], dummy[0:1, 0:8], AF.Tanh),
            inc=False)  # warms the Tanh table during the input-DMA wait

        # biases arrive as fp16 hi/lo inside blob2s; DVE reconstructs f32
        d_bias = dve([("din3", din_b2)], lambda: nc.vector.tensor_tensor(
            biases[:], bias_hi[:], bias_lo[:], op=OP.add))
        a_tsk1 = act([("dve", d_bias)],
                     lambda: act_i(tsk1buf[:], s2tbk1[:], AF.Tanh, bias=s1tk1[:, 0:1]))

        # st strips: DVE pre-adds (per-partition bias via AP scalar); the
        # first strip waits on d_bias through the semaphore - same-engine
        # RAW (biases written by the previous DVE op) is not ordered by the
        # pipeline alone
        d_sb = {}
        for i in range(NCHUNK):
            w = [("dve", d_bias)] if i == 0 else None
            d_sb[i] = dve(w, lambda i=i: nc.vector.tensor_scalar(
                stba[:, i * T:(i + 1) * T], stb[:], s1col[:, i:i + 1], None, op0=OP.add),
                inc=(i in (3, 7)))
        a_sb1 = act([("dve", d_sb[3])],
                    lambda: act_i(stbv[:, 0:4 * T], stba[:, 0:4 * T], AF.Tanh))
        # packed nv tanh: one [128, 512] pass per column half
        a_Ra = act([("pe", ga)], lambda: act_i(Rp[:, 0:512], zps[0][:, 0:512], AF.Tanh, scale=3.0))
        a_Rb = act([("pe", gb)], lambda: act_i(Rp[:, 512:1024], zps[0][:, 512:1024], AF.Tanh, scale=3.0))
        # second st panel in halves: the first rides the zstep0 window
        a_sb2a = act([("dve", d_sb[7])],
                     lambda: act_i(stbv[:, 4 * T:6 * T], stba[:, 4 * T:6 * T], AF.Tanh))

        # LtBuf = [-nv2; nv1] at the core's own rows (cols 0:1024 of Rp's
        # group 0), built in column pieces as each prep ACT completes; the
        # first 128 cols (chunk 0, j=0 rows) come first so zstep0 starts
        # as soon as possible
        def lt_piece(rows0, cs, w):
            dve(w, lambda: nc.vector.tensor_scalar_mul(
                LtBuf[rows0:rows0 + 32, cs], Rp[32:64, cs], -1.0), inc=False)
            return dve(None, lambda: nc.vector.tensor_copy(
                LtBuf[rows0 + 32:rows0 + 64, cs], Rp[0:32, cs]))

        d_Lt0n = lt_piece(0, slice(0, 128), [("act", a_Ra)])
        d_Lt0w = lt_piece(0, slice(128, 512), None)
        d_Lt0d = lt_piece(64, slice(0, 512), None)
        d_Lt1 = lt_piece(0, slice(512, 1024), [("act", a_Rb)])
        d_Lt1d = lt_piece(64, slice(512, 1024), None)

        # ts k=1 packed: relu + store (dout4); DRAM side rearranged to match
        d_tsk1 = dve([("act", a_tsk1)], lambda: nc.vector.tensor_scalar(
            tsk1buf[:], tsk1buf[:], 0.0, None, op0=OP.max))
        op("sync", [("dve", d_tsk1)],
           lambda: nc.sync.dma_start(
               out=out[NS + 128:NS + TS, 0:N].rearrange("t (b c) -> t b c", b=8),
               in_=tsk1buf[:]),
           "dout4", delta=16)

        # ---------- zsteps (rotation: step s -> zps[(s+2) % 3]) ----------
        # j = s % 2: rhs partition group / PE weight-tile row = 64*j
        zact = []
        pez = []
        z_extra = {0: [("dve", d_Lt0n)], 1: [("dve", d_Lt0d)],
                   2: [("dve", d_Lt0w)],
                   8: [("dve", d_Lt1)], 9: [("dve", d_Lt1d)]}

        # 4-deep rotation from s=6: qtt joins the cycle once the att pass
        # has drained it, so zstep s only waits on zact[s-4] - PE finishes
        # each slot ~1.1us before ACT needs it instead of ~120ns after
        SLOT6 = {6: None, 7: 2, 8: 0, 9: 1}   # None -> qtt

        def zslot(s):
            k = s if s < 6 else 6 + (s - 6) % 4
            if k >= 6:
                k2 = SLOT6[k]
                return qtt if k2 is None else zps[k2]
            return zps[(k + 2) % 3]

        def zstep(s, rs, j):
            waits = list(z_extra.get(s, []))
            if s == 6:
                waits.append(("act", att[0]))
            elif s >= 7:
                waits.append(("act", zact[s - 4]))
            elif s >= 3:
                waits.append(("act", zact[s - 3]))
            p0 = 64 * j
            lhs = LtBuf[p0:p0 + 64, rs]
            w2 = [("act", a_Rb)] if s == 0 else None
            pe(waits, lambda: mm(zslot(s)[:, 0:512], lhs, Rp[p0:p0 + 64, 0:512],
                                 start=True, stop=True))
            g = pe(w2, lambda: mm(zslot(s)[:, 512:1024], lhs,
                                  Rp[p0:p0 + 64, 512:1024], start=True, stop=True), "pe")
            pez.append(g)

        def gtt_mm(pdst, t0, tn, waits):
            if GTT_PASSES == 1:
                return pe(waits, lambda: mm(pdst, t_in["tmrT_hi"][:, t0:t0 + tn], t_in["tmT_hi"][:],
                                            start=True, stop=True), "pe")
            pe(waits, lambda: mm(pdst, t_in["tmrT_hi"][:, t0:t0 + tn], t_in["tmT_hi"][:],
                                 start=True, stop=False))
            if GTT_PASSES == 2:
                return pe(None, lambda: mm(pdst, t_in["tmrT_hi"][:, t0:t0 + tn], t_in["tmT_lo"][:],
                                           start=False, stop=True), "pe")
            pe(None, lambda: mm(pdst, t_in["tmrT_hi"][:, t0:t0 + tn], t_in["tmT_lo"][:],
                                start=False, stop=False))
            return pe(None, lambda: mm(pdst, t_in["tmrT_lo"][:, t0:t0 + tn], t_in["tmT_hi"][:],
                                       start=False, stop=True), "pe")

        outdma = []
        gtts = []
        att = []

        s = 0
        for i in range(NCHUNK):
            rs = slice(i * 128, (i + 1) * 128)
            ob = outbufs[i % 4]
            last = i == NCHUNK - 1
            for j in range(2):
                zstep(s, rs, j)
                if s == 2:
                    gtts.append(gtt_mm(qtt[0:128, 0:T], 0, 128, [("din5", din_tm)]))
                    gtts.append(gtt_mm(qtt[0:TS - 128, 512:512 + T], 128, TS - 128, None))
                ow = [(f"dout{i % 4}", outdma[i - 4])] if (j == 0 and i >= 4) else []
                if last and j == 1:
                    # split the final zact so relu+store overlap its 2nd half
                    zact.append(None)
                    za = act([("pe", pez[s])],
                             lambda ob=ob, s=s: act_i(ob[:, 1024:1536],
                                                      zslot(s)[:, 0:512],
                                                      AF.Tanh, scale=SS_C))
                    zb = act(None,
                             lambda ob=ob, s=s: act_i(ob[:, 1536:2048],
                                                      zslot(s)[:, 512:1024],
                                                      AF.Tanh, scale=SS_C))
                else:
                    zact.append(act([("pe", pez[s])] + ow,
                                    lambda ob=ob, j=j, s=s: act_i(ob[:, j * 1024:(j + 1) * 1024],
                                                                  zslot(s)[:], AF.Tanh,
                                                                  scale=SS_C)))
                s += 1
                # temporal k=0 ts region rides the first zact slack window
                if s == 2:
                    a_ts = act([("din6", din_sF)],
                               lambda: act_i(tob0[:, 0:N], s2tbF[:], AF.Tanh,
                                             bias=s1tcol[:, 0:1]), inc=False)
            if last:
                # tail: ss pieces relu+store independently (st strip is part
                # of the stbv panel, stored mid-pipeline)
                d_a = dve([("act", zact[2 * i])], lambda ob=ob: nc.vector.tensor_scalar(
                    ob[:, 0:1024], ob[:, 0:1024], 0.0, SS_S, op0=OP.max, op1=OP.mult))
                op("sync", [("dve", d_a)],
                   lambda ob=ob, rs=rs: nc.sync.dma_start(out=out[rs, 0:1024],
                                                          in_=ob[:, 0:1024]),
                   f"dout{i % 4}", delta=16)
                d_b1 = dve([("act", za)], lambda ob=ob: nc.vector.tensor_scalar(
                    ob[:, 1024:1536], ob[:, 1024:1536], 0.0, SS_S, op0=OP.max, op1=OP.mult))
                op("sync", [("dve", d_b1)],
                   lambda ob=ob, rs=rs: nc.sync.dma_start(out=out[rs, 1024:1536],
                                                          in_=ob[:, 1024:1536]),
                   f"dout{i % 4}", delta=16)
                d_b2 = dve([("act", zb)], lambda ob=ob: nc.vector.tensor_scalar(
                    ob[:, 1536:2048], ob[:, 1536:2048], 0.0, SS_S, op0=OP.max, op1=OP.mult))
                # final store triggers from the now-idle Scalar hwdge queue
                # so it doesn't serialize behind the previous store on Sync
                op("scalar", [("dve", d_b2)],
                   lambda ob=ob, rs=rs: nc.scalar.dma_start(out=out[rs, 1536:2048],
                                                            in_=ob[:, 1536:2048]),
                   f"dout{i % 4}", delta=16)
            else:
                d_ss = dve([("act", zact[2 * i + 1])], lambda ob=ob: nc.vector.tensor_scalar(
                    ob[:], ob[:], 0.0, SS_S, op0=OP.max, op1=OP.mult))
                outdma.append(op("sync", [("dve", d_ss)],
                                 lambda ob=ob, rs=rs: nc.sync.dma_start(out=out[rs, 0:N], in_=ob[:]),
                                 f"dout{i % 4}", delta=16))
            if i == 0:
                a_sb2b = act([("dve", d_sb[7])],
                             lambda: act_i(stbv[:, 6 * T:8 * T], stba[:, 6 * T:8 * T], AF.Tanh))
            elif i == 1:
                # st panel: relu once, store all 8 strips with one DMA
                d_sv = dve([("act", a_sb2b)], lambda: nc.vector.tensor_scalar(
                    stbv[:], stbv[:], 0.0, None, op0=OP.max))
                op("sync", [("dve", d_sv)],
                   lambda: nc.sync.dma_start(
                       out=out[0:NS, N:NT].rearrange("(i p) t -> p i t", p=128),
                       in_=stbv[:].rearrange("p (i t) -> p i t", t=T)),
                   "dout4", delta=16)
            # interleave temporal ACT work into the stream
            if i == 1:
                # both tt row groups tanh'd in one pass via a strided PSUM
                # AP; emitted here so the s=6 qtt-reuse gate clears before
                # the PE reaches it
                att.append(act([("pe", gtts[1])], lambda: act_i(
                    tttbufM[:], qtt[:, 0:1024].rearrange("p (b c) -> p b c", b=2)[:, :, 0:T],
                    AF.Tanh)))
            elif i == 4:
                # temporal k=0 rows: mask tt, relu, store whole [128, 2336]
                dmm = dve([("act", att[0]), ("din4", din_masks)], lambda: nc.vector.tensor_tensor(
                    tob0[:, N:NT], tttbufM[:, 0:T], mask0[:], op=OP.mult))
                dr = dve([("dve", dmm)], lambda: nc.vector.tensor_scalar(
                    tob0[:], tob0[:], 0.0, None, op0=OP.max))
                op("sync", [("dve", dr)],
                   lambda: nc.sync.dma_start(out=out[NS:NS + 128, :], in_=tob0[:]),
                   "dout4", delta=16)
                # temporal k=1 tt region [16, 288]
                dm1 = dve(None, lambda: nc.vector.tensor_tensor(
                    ttk1buf[:], tttbufM[0:TS - 128, T:2 * T], mask1[:], op=OP.mult))
                dr1 = dve([("dve", dm1)], lambda: nc.vector.tensor_scalar(
                    ttk1buf[:], ttk1buf[:], 0.0, None, op0=OP.max))
                op("sync", [("dve", dr1)],
                   lambda: nc.sync.dma_start(out=out[NS + 128:NS + TS, N:NT], in_=ttk1buf[:]),
                   "dout4", delta=16)

        # ---------- emit (waits embedded into the consuming instruction) ---
        with nc.Block() as block:
            def make_body(engine_name):
                ops = plan[engine_name]

                # waits can be embedded only on single-instruction ops: a
                # matmul emits LDWEIGHTS before the MATMUL that would carry
                # the wait (weights read unguarded), and DMA triggers may
                # start descriptor generation early - both need standalone
                # event-sem waits ahead of the op
                embed_ok = engine_name in ("scalar", "vector", "tensor", "sync")

                def body(eng):
                    satisfied = {}
                    for waits, fn, inc in ops:
                        pend = []
                        for sem_name, val in waits:
                            if val is not None and satisfied.get(sem_name, -1) < val:
                                pend.append((sem_name, val))
                                satisfied[sem_name] = val
                        tail = pend[1:] if embed_ok else pend
                        for sem_name, val in tail:
                            eng.wait_ge(SEM[sem_name], val)
                        ins = fn()
                        if embed_ok and pend:
                            ins.wait_op(SEM[pend[0][0]], pend[0][1], "sem-ge")
                        if inc is None:
                            continue
                        if inc.startswith("din") or inc.startswith("dout"):
                            ins.then_inc(SEM[inc], 16)
                        else:
                            ins.then_inc(SEM[inc], 1)
                return body

            block.sync(make_body("sync"))
            block.tensor(make_body("tensor"))
            block.scalar(make_body("scalar"))
            block.vector(make_body("vector"))

    return nc


def _hilo(a):
    hi = a.astype(np.float16)
    lo = (a - hi.astype(np.float32)).astype(np.float16)
    return hi, lo


def build_in_maps(spatial_nodes, temporal_nodes, W_ss1, W_ss2, w_st, b_st, w_ts, b_ts):
    f = np.float32
    h16 = np.float16
    W12T = np.concatenate([W_ss1.T, W_ss2.T], axis=1).astype(f)
    W_hi, W_lo = _hilo(W12T)
    in_maps = []
    for c in range(N_CORES):
        b, hh = divmod(c, 2)
        tmask = (np.arange(T)[None, :] >= (hh * TS + np.arange(TS))[:, None]).astype(h16)
        # rotate spatial columns so this core's row-half sits at cols 0:NS
        spT = np.ascontiguousarray(np.roll(spatial_nodes[b].T, -hh * NS, axis=1), dtype=f)
        tmT = np.ascontiguousarray(temporal_nodes[b].T, dtype=f)
        sp_hi, sp_lo = _hilo(spT)
        tm_hi, tm_lo = _hilo(tmT)
        # block order [A=0:512, C=1024:1536, B=512:1024, D=1536:2048] for
        # hi then lo - matches the kernel's POS map and progressive DMAs
        blob1 = np.empty((D, B1_W), h16)
        blob1[:, 0:2 * D] = W_hi
        blob1[:, 2 * D:4 * D] = W_lo
        for p, s0 in enumerate((0, 1024, 512, 1536)):
            blob1[:, 4 * D + p * 512:4 * D + (p + 1) * 512] = sp_hi[:, s0:s0 + 512]
            blob1[:, 4 * D + N + p * 512:4 * D + N + (p + 1) * 512] = sp_lo[:, s0:s0 + 512]
        partsT = {
            "tmT_hi": tm_hi, "tmT_lo": tm_lo,
            "tmrT_hi": tm_hi[:, hh * TS:(hh + 1) * TS],
            "tmrT_lo": tm_lo[:, hh * TS:(hh + 1) * TS],
        }
        tmblob = np.empty((D, TM_W), h16)
        for nm, c0, c1 in TM_SLICES:
            tmblob[:, c0:c1] = partsT[nm]
        # host-side small linear transforms (same class as transpose/hi-lo prep)
        s1 = spT[:, 0:NS].T @ w_st[:D].astype(f)             # [NS]
        s2 = (temporal_nodes[b] @ w_st[D:].astype(f)) + f(b_st)   # [T]
        s1t = temporal_nodes[b, hh * TS:(hh + 1) * TS] @ w_ts[:D].astype(f)  # [TS]
        s2t = spT.T @ w_ts[D:].astype(f) + f(b_ts)           # [N] rotated order
        # k=1 packed: row t*8+blk holds s2t[blk*256 : blk*256+256]
        s2tbk1 = np.ascontiguousarray(s2t.astype(h16).reshape(8, 256)[
            np.tile(np.arange(8), 16), :])
        s2tbF = np.broadcast_to(s2t.astype(h16).ravel(), (128, N)).copy()
        biases = np.zeros((128, NBIAS), f)
        biases[:, 0:NCHUNK] = s1.reshape(NCHUNK, 128).T
        biases[0:128, NCHUNK] = s1t[0:128]
        biases[0:TS - 128, NCHUNK + 1] = s1t[128:TS]
        biases[:, NCHUNK + 2] = np.repeat(s1t[128:TS], 8)
        b_hi, b_lo = _hilo(biases)
        blob2s = np.empty((128, B2S_W), h16)
        blob2s[:, 0:T] = s2.astype(h16)[None, :]
        blob2s[:, T:T + 256] = s2tbk1
        blob2s[:, T + 256:T + 256 + NBIAS] = b_hi
        blob2s[:, T + 256 + NBIAS:] = b_lo
        in_maps.append({
            "blob1": blob1,
            "blob2s": blob2s,
            "s2tbF": s2tbF,
            "tmblob": tmblob,
            "ttmask": tmask,
        })
    return in_maps


def assemble(results):
    out = np.empty((B, NT, NT), np.float32)
    for c in range(N_CORES):
        b, h = divmod(c, 2)
        r = results[c]["out"].astype(np.float32)
        # un-rotate spatial columns (host rotated by -h*NS)
        sp_cols = np.roll(r[:, 0:N], h * NS, axis=1)
        out[b, h * NS:(h + 1) * NS, 0:N] = sp_cols[0:NS]
        out[b, h * NS:(h + 1) * NS, N:NT] = r[0:NS, N:NT]
        out[b, N + h * TS: N + (h + 1) * TS, 0:N] = sp_cols[NS:ROWS]
        out[b, N + h * TS: N + (h + 1) * TS, N:NT] = r[NS:ROWS, N:NT]
    return out


_NC = None


def kernel(**inputs):
    global _NC
    if _NC is None:
        _NC = build_program()
    in_maps = build_in_maps(**inputs)
    res = run_bass_kernel_spmd(_NC, in_maps, list(range(N_CORES)))
    return assemble(res.results)


# revision 45
# speedup vs baseline: 1.0276x; 1.0126x over previous
"""Trainium2 Bass kernel for the MLPSim adjacency-constructor problem.

Full shapes: spatial [4, 2048, 32], temporal [4, 288, 32], output
adj [4, 2336, 2336] f32 where adj = tanh(relu(blocks)):
  ss = tanh(m - m^T), m = nv1 @ nv2^T, nv_i = tanh(3*x@W_i^T)
  st = s1[n] + s2[t] + b_st ;  ts = s1t[t] + s2t[n] + b_ts
  tt = triu(temporal @ temporal^T)

Sharding: 8 cores = (batch b = c//2) x (row-half h = c%2); each core emits
1024 spatial + 144 temporal rows ([1168, 2336]) of one batch. Spatial
COLUMNS are rotated by -h*1024 on the host so each core's row-half sits at
columns 0:1024 (assembly un-rotates).

Device algebra (ACT-bound design, fp16 datapath):
  ss: tanh(relu(tanh(d))) ~= S*relu(tanh(C*d)), minimax S=0.7552623
      C=1.2825139 (maxerr 6.3e-3) -> ONE ACT tanh pass from PSUM + ONE
      DVE dual-op tensor_scalar (max 0, mult S) at 4x fp16 mode.
  Prep z = x@W packs BOTH spatial column-halves into one [128,1024] PSUM
  tile (cols 0:1024 in partitions 0:64, cols 1024:2048 in 64:128); the
  two partition groups use PE weight tiles (0,*) and (0,64) and their
  matmuls overlap on the array, and the nv tanh is 2 ACT passes at half
  the columns. Rp keeps the packed layout; zstep j reads rhs from
  partitions 64j:64j+64 with the weight tile at row 64j (lhsT always
  LtBuf[0:64] = [-nv2; nv1], loaded via explicit tile_position). d
  accumulates in K=64 fp16 matmuls; the PSUM rotation is 3-deep for
  s<6 and 4-deep after (qtt joins the cycle once the att pass drains
  it, emitted one chunk early so the s=6 gate clears in time) - with
  zstep s waiting zact[s-4] the PE fills each slot ~1.1us before ACT
  needs it and the zact phase runs gap-free; do NOT merge adjacent-slot
  zact pairs into one [128,2048] pass (holding two slots starves the PE
  and costs ~1.1us/chunk). prep/tt matmuls use fp16 hi/lo splitting
  (accumulating
  passes at 1 cyc/col), all hi passes first so the late-landing lo blob
  overlaps them. Broadcast/linear tiles (stb, s2tbF, biases as fp16
  hi/lo pairs) are host inputs; no PE broadcasts. Input DMA triggers
  issue from the Sync hwdge queue critical-consumer first, plus one on
  the idle Scalar queue (gpsimd software-DGE triggers measured ~3us
  issue latency - do not use). The st block is built by DVE bias
  pre-adds into one [128, 8*288] strip, tanh'd in three ACT passes,
  relu'd once, and stored with a single rearranged DMA. Temporal ts
  rows are one fused [128,2048] ACT pass in zact slack; both tt row
  groups share one ACT pass via a strided 2-bank PSUM AP. Semaphore
  waits are embedded in the consuming instruction (separate
  EVENT_SEMAPHORE ops cost ~100ns each on the queue). The tail splits
  the last zact into 512-col pieces so relu/store overlap it. All
  stores fp16; host assembles and upcasts.
  Runtime pitfalls baked in: fp32 matmuls run as 2 half-speed passes
  (fp32r is verifier-trapped; fp16 hi/lo instead); 1-partition matmuls
  need their own PSUM bank; same-engine RAW needs a semaphore even
  between adjacent queue entries (biases reconstruct vs first consumer
  raced without one); a DMA semaphore with multiple writers only
  supports all-or-nothing waits; fmap and weights must share the SB
  base partition (hence the LtBuf base-64 duplicate); tanh-degree-11
  polynomials on DVE lose to fp16 intermediate rounding (2.8e-2) - not
  a viable ACT offload.
"""

import numpy as np
from contextlib import ExitStack

import concourse.bass as bass
from concourse import mybir
from concourse.bass_utils import run_bass_kernel_spmd

AF = mybir.ActivationFunctionType
OP = mybir.AluOpType
F32 = mybir.dt.float32
F16 = mybir.dt.float16

B, N, T, D = 4, 2048, 288, 32
NS = N // 2          # 1024 spatial rows per core
TS = T // 2          # 144 temporal rows per core
NT = N + T           # 2336
ROWS = NS + TS       # 1168
N_CORES = 8
NCHUNK = NS // 128   # 8 spatial row-chunks

SS_S = 0.7552623    # tanh(relu(tanh(d))) ~= SS_S * relu(tanh(SS_C*d))
SS_C = 1.2825139

PREP_PASSES = 3      # hi@Whi, hi@Wlo, lo@Whi
GTT_PASSES = 3       # hi.hi, hi.lo, lo.hi

# blob1 [D, B1_W] fp16: prep inputs (W transposed pair, spatial hi/lo)
_B1_DEFS = (("W12T_hi", 2 * D), ("W12T_lo", 2 * D), ("spT_hi", N), ("spT_lo", N))
B1_SLICES = []
_c = 0
for _nm, _w in _B1_DEFS:
    B1_SLICES.append((_nm, _c, _c + _w))
    _c += _w
B1_W = _c

# tmblob [D, TM_W] fp16: temporal hi/lo for the tt matmuls
_TM_DEFS = (("tmT_hi", T), ("tmT_lo", T), ("tmrT_hi", TS), ("tmrT_lo", TS))
TM_SLICES = []
_c = 0
for _nm, _w in _TM_DEFS:
    TM_SLICES.append((_nm, _c, _c + _w))
    _c += _w
TM_W = _c

# blob2s [128, B2S_W] fp16: small early broadcast tiles
#   stb[p, t] = s2[t] + b_st, s2tbk1[t*8+b, c] = s2t[b*256+c],
#   biases packed as fp16 hi/lo pairs (DVE reconstructs f32 on device)
NBIAS = NCHUNK + 3
B2S_W = T + 256 + 2 * NBIAS


def build_program():
    nc = bass.Bass()
    inp = {}

    def di(name, shape, dt=F16):
        inp[name] = nc.declare_dram_parameter(name, list(shape), dt, isOutput=False)

    di("blob1", (D, B1_W))
    di("blob2s", (128, B2S_W))
    di("s2tbF", (128, N))          # host bcast: s2t + b_ts (rotated col order)
    di("tmblob", (D, TM_W))
    di("ttmask", (TS, T))
    out = nc.declare_dram_parameter("out", [ROWS, NT], F16, isOutput=True)

    ctx = ExitStack()
    _uid = [0]

    def sbuf(shape, dt=F16):
        _uid[0] += 1
        return ctx.enter_context(nc.sbuf_tensor(f"sb{_uid[0]}", shape, dt))

    def psum(shape):
        _uid[0] += 1
        return ctx.enter_context(nc.psum_tensor(f"ps{_uid[0]}", shape, F32))

    with ctx:
        blob1 = sbuf([D, B1_W])
        t_in = {nm: blob1[:, c0:c1] for nm, c0, c1 in B1_SLICES}
        tmblob = sbuf([D, TM_W])
        for nm, c0, c1 in TM_SLICES:
            t_in[nm] = tmblob[:, c0:c1]
        blob2s = sbuf([128, B2S_W])
        stb = blob2s[:, 0:T]
        s2tbk1 = blob2s[:, T:T + 256]
        bias_hi = blob2s[:, T + 256:T + 256 + NBIAS]
        bias_lo = blob2s[:, T + 256 + NBIAS:T + 256 + 2 * NBIAS]
        s2tbF = sbuf([128, N])
        dummy = sbuf([1, 16])
        biases = sbuf([128, NBIAS], F32)
        s1col = biases[:, 0:NCHUNK]
        s1tcol = biases[:, NCHUNK:NCHUNK + 2]
        s1tk1 = biases[:, NCHUNK + 2:NCHUNK + 3]
        mask0 = sbuf([128, T])
        mask1 = sbuf([TS - 128, T])
        # packed nv: partitions 0:64 = [nv1; nv2] for spatial cols 0:1024,
        # partitions 64:128 = same for cols 1024:2048
        Rp = sbuf([128, 1024])
        # lhsT: rows 0:32 = -nv2, rows 32:64 = nv1 (at the chunk's rows);
        # rows 64:128 duplicate rows 0:64 (fmap and weights must share the
        # SB base partition, so the j=1 weight tile needs a base-64 copy)
        LtBuf = sbuf([128, NS])
        stba = sbuf([128, NCHUNK * T])       # st strips: stb + s1col[i]
        stbv = sbuf([128, NCHUNK * T])       # tanh(st strips)
        outbufs = [sbuf([128, N]) for _ in range(4)]
        tob0 = sbuf([128, NT])               # temporal k=0 rows
        tsk1buf = sbuf([128, 256])           # temporal k=1 ts region, packed
        ttk1buf = sbuf([TS - 128, T])        # temporal k=1 tt region
        tttbufM = sbuf([128, 2 * T])         # tt tanh: k=0 | k=1 halves

        zps = [psum([128, 1024]) for _ in range(3)]   # 6 banks
        # gtt: k=0 rows in cols 0:T (bank 6), k=1 rows in 512:512+T (bank 7)
        qtt = psum([128, 1024])

        dmain = ctx.enter_context(nc.semaphore("dmain"))
        dmain2 = ctx.enter_context(nc.semaphore("dmain2"))
        dmain3 = ctx.enter_context(nc.semaphore("dmain3"))
        dmain4 = ctx.enter_context(nc.semaphore("dmain4"))
        dmain5 = ctx.enter_context(nc.semaphore("dmain5"))
        dmain6 = ctx.enter_context(nc.semaphore("dmain6"))
        dmain7 = ctx.enter_context(nc.semaphore("dmain7"))
        pe_s = ctx.enter_context(nc.semaphore("pe_s"))
        act_s = ctx.enter_context(nc.semaphore("act_s"))
        dve_s = ctx.enter_context(nc.semaphore("dve_s"))
        douts = [ctx.enter_context(nc.semaphore(f"dout{k}")) for k in range(5)]
        SEM = {"pe": pe_s, "act": act_s, "dve": dve_s, "din": dmain,
               "din2": dmain2, "din3": dmain3, "din4": dmain4, "din5": dmain5,
               "din6": dmain6, "din7": dmain7,
               "dout0": douts[0], "dout1": douts[1], "dout2": douts[2],
               "dout3": douts[3], "dout4": douts[4]}

        plan = {"sync": [], "tensor": [], "scalar": [], "vector": [], "gpsimd": []}
        cnt = {"pe": 0, "act": 0, "dve": 0, "din": 0, "din2": 0, "din3": 0,
               "din4": 0, "din5": 0, "din6": 0, "din7": 0,
               "dout0": 0, "dout1": 0, "dout2": 0, "dout3": 0, "dout4": 0}

        def op(engine, waits, fn, inc=None, delta=None):
            plan[engine].append((waits or [], fn, inc))
            if inc:
                if delta is None:
                    delta = 16 if inc.startswith("d") and inc != "dve" else 1
                cnt[inc] += delta
                return cnt[inc]
            return None

        # ---------- input loads: sync hwdge queue, priority order ----------
        # blob1 arrives as 3 progressive DMAs; host orders spatial blocks
        # [A=0:512, C=1024:1536, B=512:1024, D=1536:2048] for hi then lo,
        # so the first DMA (W + A + C) feeds the first matmul pair at once
        B1H1 = 2 * (2 * D) + 1024
        B1H2 = 2 * (2 * D) + N
        op("sync", None, lambda: nc.sync.dma_start(out=blob1[:, 0:B1H1], in_=inp["blob1"][:, 0:B1H1]), "din", delta=16)
        din_b1 = cnt["din"]
        # blob2s rides the Scalar hwdge queue (idle until the table load)
        op("scalar", None, lambda: nc.scalar.dma_start(out=blob2s[:], in_=inp["blob2s"][:]), "din3", delta=16)
        din_b2 = cnt["din3"]
        op("sync", None, lambda: nc.sync.dma_start(out=blob1[:, B1H1:B1H2], in_=inp["blob1"][:, B1H1:B1H2]), "din7", delta=16)
        din_b1h2 = cnt["din7"]
        op("sync", None, lambda: nc.sync.dma_start(out=blob1[:, B1H2:B1_W], in_=inp["blob1"][:, B1H2:B1_W]), "din2", delta=16)
        din_b1lo = cnt["din2"]
        op("sync", None, lambda: nc.sync.dma_start(out=s2tbF[:], in_=inp["s2tbF"][:]), "din6", delta=16)
        din_sF = cnt["din6"]
        op("sync", None, lambda: nc.sync.dma_start(out=tmblob[:], in_=inp["tmblob"][:]), "din5", delta=16)
        din_tm = cnt["din5"]
        op("sync", None, lambda: nc.sync.dma_start(out=mask0[:], in_=inp["ttmask"][0:128, :]), "din4", delta=16)
        op("sync", None, lambda: nc.sync.dma_start(out=mask1[:], in_=inp["ttmask"][128:TS, :]), "din4", delta=16)
        din_masks = cnt["din4"]

        Whi, Wlo = t_in["W12T_hi"], t_in["W12T_lo"]
        mm = nc.tensor.matmul
        act_i = nc.scalar.activation

        def pe(waits, fn, inc=None):
            return op("tensor", waits, fn, inc)

        def act(waits, fn, inc=True):
            return op("scalar", waits, fn, "act" if inc else None)

        def dve(waits, fn, inc=True):
            return op("vector", waits, fn, "dve" if inc else None)

        # ---------- nv prep: z = x@W, hi/lo accumulate, 128-part packed ----
        # partition group g covers spatial cols [g*1024, (g+1)*1024) at psum
        # cols = local col; both groups' matmuls overlap on the PE array.
        # blob1 block layout: hi blocks [A, C, B, D] at B1H0+pos*512, lo
        # blocks in the same order afterwards; PSUM region (g, cc) maps to
        # pos {(0,0):0, (1,0):1, (0,512):2, (1,512):3} so each (A,C)/(B,D)
        # pair overlaps as two PE weight tiles
        B1H0 = 2 * (2 * D)
        B1LO = B1H0 + N
        POS = {(0, 0): 0, (1, 0): 1, (0, 512): 2, (1, 512): 3}

        ga = gb = None
        for pi, (W_, base) in enumerate(((Whi, B1H0), (Wlo, B1H0), (Whi, B1LO))):
            for cc in (0, 512):
                for g in range(2):
                    dst = zps[0][64 * g:64 * (g + 1), cc:cc + 512]
                    c0 = base + POS[(g, cc)] * 512
                    w = None
                    if g == 0 and pi == 0:
                        w = [("din", din_b1)] if cc == 0 else [("din7", din_b1h2)]
                    elif g == 0 and cc == 0 and pi == 2:
                        w = [("din2", din_b1lo)]
                    gend = pe(w, lambda dst=dst, W_=W_, c0=c0, pi=pi: mm(
                        dst, W_[:], blob1[:, c0:c0 + 512],
                        start=pi == 0, stop=pi == 2),
                        "pe" if pi == 2 else None)
                if pi == 2 and cc == 0:
                    ga = gend
                elif pi == 2:
                    gb = gend

        # ---------- ACT stream: warm, early fill, prep tanh, zacts ---------
        d_dum = dve(None, lambda: nc.vector.memset(dummy[:], 0.25))
        act([("dve", d_dum)], lambda: act_i(dummy[0:1, 8:16], dummy[0:1, 0:8], AF.Tanh),
            inc=False)  # warms the Tanh table during the input-DMA wait

        # biases arrive as fp16 hi/lo inside blob2s; DVE reconstructs f32
        d_bias = dve([("din3", din_b2)], lambda: nc.vector.tensor_tensor(
            biases[:], bias_hi[:], bias_lo[:], op=OP.add))
        a_tsk1 = act([("dve", d_bias)],
                     lambda: act_i(tsk1buf[:], s2tbk1[:], AF.Tanh, bias=s1tk1[:, 0:1]))

        # st strips: DVE pre-adds (per-partition bias via AP scalar); the
        # first strip waits on d_bias through the semaphore - same-engine
        # RAW (biases written by the previous DVE op) is not ordered by the
        # pipeline alone
        d_sb = {}
        for i in range(NCHUNK):
            w = [("dve", d_bias)] if i == 0 else None
            d_sb[i] = dve(w, lambda i=i: nc.vector.tensor_scalar(
                stba[:, i * T:(i + 1) * T], stb[:], s1col[:, i:i + 1], None, op0=OP.add),
                inc=(i in (3, 7)))
        a_sb1 = act([("dve", d_sb[3])],
                    lambda: act_i(stbv[:, 0:4 * T], stba[:, 0:4 * T], AF.Tanh))
        # packed nv tanh: one [128, 512] pass per column half
        a_Ra = act([("pe", ga)], lambda: act_i(Rp[:, 0:512], zps[0][:, 0:512], AF.Tanh, scale=3.0))
        a_Rb = act([("pe", gb)], lambda: act_i(Rp[:, 512:1024], zps[0][:, 512:1024], AF.Tanh, scale=3.0))
        # second st panel in halves: the first rides the zstep0 window
        a_sb2a = act([("dve", d_sb[7])],
                     lambda: act_i(stbv[:, 4 * T:6 * T], stba[:, 4 * T:6 * T], AF.Tanh))

        # LtBuf = [-nv2; nv1] at the core's own rows (cols 0:1024 of Rp's
        # group 0), built in column pieces as each prep ACT completes; the
        # first 128 cols (chunk 0, j=0 rows) come first so zstep0 starts
        # as soon as possible
        def lt_piece(rows0, cs, w):
            dve(w, lambda: nc.vector.tensor_scalar_mul(
                LtBuf[rows0:rows0 + 32, cs], Rp[32:64, cs], -1.0), inc=False)
            return dve(None, lambda: nc.vector.tensor_copy(
                LtBuf[rows0 + 32:rows0 + 64, cs], Rp[0:32, cs]))

        d_Lt0n = lt_piece(0, slice(0, 128), [("act", a_Ra)])
        d_Lt0w = lt_piece(0, slice(128, 512), None)
        d_Lt0d = lt_piece(64, slice(0, 512), None)
        d_Lt1 = lt_piece(0, slice(512, 1024), [("act", a_Rb)])
        d_Lt1d = lt_piece(64, slice(512, 1024), None)

        # ts k=1 packed: relu + store (dout4); DRAM side rearranged to match
        d_tsk1 = dve([("act", a_tsk1)], lambda: nc.vector.tensor_scalar(
            tsk1buf[:], tsk1buf[:], 0.0, None, op0=OP.max))
        op("sync", [("dve", d_tsk1)],
           lambda: nc.sync.dma_start(
               out=out[NS + 128:NS + TS, 0:N].rearrange("t (b c) -> t b c", b=8),
               in_=tsk1buf[:]),
           "dout4", delta=16)

        # ---------- zsteps (rotation: step s -> zps[(s+2) % 3]) ----------
        # j = s % 2: rhs partition group / PE weight-tile row = 64*j
        zact = []
        pez = []
        z_extra = {0: [("dve", d_Lt0n)], 1: [("dve", d_Lt0d)],
                   2: [("dve", d_Lt0w)],
                   8: [("dve", d_Lt1)], 9: [("dve", d_Lt1d)]}

        # 4-deep rotation from s=6: qtt joins the cycle once the att pass
        # has drained it, so zstep s only waits on zact[s-4] - PE finishes
        # each slot ~1.1us before ACT needs it instead of ~120ns after
        SLOT6 = {6: None, 7: 2, 8: 0, 9: 1}   # None -> qtt

        def zslot(s):
            k = s if s < 6 else 6 + (s - 6) % 4
            if k >= 6:
                k2 = SLOT6[k]
                return qtt if k2 is None else zps[k2]
            return zps[(k + 2) % 3]

        def zstep(s, rs, j):
            waits = list(z_extra.get(s, []))
            if s == 6:
                waits.append(("act", att[0]))
            elif s >= 7:
                waits.append(("act", zact[s - 4]))
            elif s >= 3:
                waits.append(("act", zact[s - 3]))
            p0 = 64 * j
            lhs = LtBuf[p0:p0 + 64, rs]
            w2 = [("act", a_Rb)] if s == 0 else None
            pe(waits, lambda: mm(zslot(s)[:, 0:512], lhs, Rp[p0:p0 + 64, 0:512],
                                 start=True, stop=True))
            g = pe(w2, lambda: mm(zslot(s)[:, 512:1024], lhs,
                                  Rp[p0:p0 + 64, 512:1024], start=True, stop=True), "pe")
            pez.append(g)

        def gtt_mm(pdst, t0, tn, waits):
            if GTT_PASSES == 1:
                return pe(waits, lambda: mm(pdst, t_in["tmrT_hi"][:, t0:t0 + tn], t_in["tmT_hi"][:],
                                            start=True, stop=True), "pe")
            pe(waits, lambda: mm(pdst, t_in["tmrT_hi"][:, t0:t0 + tn], t_in["tmT_hi"][:],
                                 start=True, stop=False))
            if GTT_PASSES == 2:
                return pe(None, lambda: mm(pdst, t_in["tmrT_hi"][:, t0:t0 + tn], t_in["tmT_lo"][:],
                                           start=False, stop=True), "pe")
            pe(None, lambda: mm(pdst, t_in["tmrT_hi"][:, t0:t0 + tn], t_in["tmT_lo"][:],
                                start=False, stop=False))
            return pe(None, lambda: mm(pdst, t_in["tmrT_lo"][:, t0:t0 + tn], t_in["tmT_hi"][:],
                                       start=False, stop=True), "pe")

        outdma = []
        gtts = []
        att = []

        s = 0
        for i in range(NCHUNK):
            rs = slice(i * 128, (i + 1) * 128)
            ob = outbufs[i % 4]
            last = i == NCHUNK - 1
            for j in range(2):
                zstep(s, rs, j)
                if s == 2:
                    gtts.append(gtt_mm(qtt[0:128, 0:T], 0, 128, [("din5", din_tm)]))
                    gtts.append(gtt_mm(qtt[0:TS - 128, 512:512 + T], 128, TS - 128, None))
                ow = [(f"dout{i % 4}", outdma[i - 4])] if (j == 0 and i >= 4) else []
                if last and j == 1:
                    # split the final zact so relu+store overlap its 2nd half
                    zact.append(None)
                    za = act([("pe", pez[s])],
                             lambda ob=ob, s=s: act_i(ob[:, 1024:1536],
                                                      zslot(s)[:, 0:512],
                                                      AF.Tanh, scale=SS_C))
                    zb = act(None,
                             lambda ob=ob, s=s: act_i(ob[:, 1536:2048],
                                                      zslot(s)[:, 512:1024],
                                                      AF.Tanh, scale=SS_C))
                else:
                    zact.append(act([("pe", pez[s])] + ow,
                                    lambda ob=ob, j=j, s=s: act_i(ob[:, j * 1024:(j + 1) * 1024],
                                                                  zslot(s)[:], AF.Tanh,
                                                                  scale=SS_C)))
                s += 1
                # temporal k=0 ts region rides the first zact slack window
                if s == 2:
                    a_ts = act([("din6", din_sF)],
                               lambda: act_i(tob0[:, 0:N], s2tbF[:], AF.Tanh,
                                             bias=s1tcol[:, 0:1]), inc=False)
            if last:
                # tail: ss pieces relu+store independently (st strip is part
                # of the stbv panel, stored mid-pipeline)
                d_a = dve([("act", zact[2 * i])], lambda ob=ob: nc.vector.tensor_scalar(
                    ob[:, 0:1024], ob[:, 0:1024], 0.0, SS_S, op0=OP.max, op1=OP.mult))
                op("sync", [("dve", d_a)],
                   lambda ob=ob, rs=rs: nc.sync.dma_start(out=out[rs, 0:1024],
                                                          in_=ob[:, 0:1024]),
                   f"dout{i % 4}", delta=16)
                d_b1 = dve([("act", za)], lambda ob=ob: nc.vector.tensor_scalar(
                    ob[:, 1024:1536], ob[:, 1024:1536], 0.0, SS_S, op0=OP.max, op1=OP.mult))
                op("sync", [("dve", d_b1)],
                   lambda ob=ob, rs=rs: nc.sync.dma_start(out=out[rs, 1024:1536],
                                                          in_=ob[:, 1024:1536]),
                   f"dout{i % 4}", delta=16)
                d_b2 = dve([("act", zb)], lambda ob=ob: nc.vector.tensor_scalar(
                    ob[:, 1536:2048], ob[:, 1536:2048], 0.0, SS_S, op0=OP.max, op1=OP.mult))
                # final store triggers from the now-idle Scalar hwdge queue
                # so it doesn't serialize behind the previous store on Sync
                op("scalar", [("dve", d_b2)],
                   lambda ob=ob, rs=rs: nc.scalar.dma_start(out=out[rs, 1536:2048],
                                                            in_=ob[:, 1536:2048]),
                   f"dout{i % 4}", delta=16)
            else:
                d_ss = dve([("act", zact[2 * i + 1])], lambda ob=ob: nc.vector.tensor_scalar(
                    ob[:], ob[:], 0.0, SS_S, op0=OP.max, op1=OP.mult))
                outdma.append(op("sync", [("dve", d_ss)],
                                 lambda ob=ob, rs=rs: nc.sync.dma_start(out=out[rs, 0:N], in_=ob[:]),
                                 f"dout{i % 4}", delta=16))
            if i == 0:
                a_sb2b = act([("dve", d_sb[7])],
                             lambda: act_i(stbv[:, 6 * T:8 * T], stba[:, 6 * T:8 * T], AF.Tanh))
            elif i == 1:
                # st panel: relu once, store all 8 strips with one DMA
                d_sv = dve([("act", a_sb2b)], lambda: nc.vector.tensor_scalar(
                    stbv[:], stbv[:], 0.0, None, op0=OP.max))
                op("sync", [("dve", d_sv)],
                   lambda: nc.sync.dma_start(
                       out=out[0:NS, N:NT].rearrange("(i p) t -> p i t", p=128),
                       in_=stbv[:].rearrange("p (i t) -> p i t", t=T)),
                   "dout4", delta=16)
            # interleave temporal ACT work into the stream
            if i == 1:
                # both tt row groups tanh'd in one pass via a strided PSUM
                # AP; emitted here so the s=6 qtt-reuse gate clears before
                # the PE reaches it
                att.append(act([("pe", gtts[1])], lambda: act_i(
                    tttbufM[:], qtt[:, 0:1024].rearrange("p (b c) -> p b c", b=2)[:, :, 0:T],
                    AF.Tanh)))
            elif i == 4:
                # temporal k=0 rows: mask tt, relu, store whole [128, 2336]
                dmm = dve([("act", att[0]), ("din4", din_masks)], lambda: nc.vector.tensor_tensor(
                    tob0[:, N:NT], tttbufM[:, 0:T], mask0[:], op=OP.mult))
                dr = dve([("dve", dmm)], lambda: nc.vector.tensor_scalar(
                    tob0[:], tob0[:], 0.0, None, op0=OP.max))
                op("sync", [("dve", dr)],
                   lambda: nc.sync.dma_start(out=out[NS:NS + 128, :], in_=tob0[:]),
                   "dout4", delta=16)
                # temporal k=1 tt region [16, 288]
                dm1 = dve(None, lambda: nc.vector.tensor_tensor(
                    ttk1buf[:], tttbufM[0:TS - 128, T:2 * T], mask1[:], op=OP.mult))
                dr1 = dve([("dve", dm1)], lambda: nc.vector.tensor_scalar(
                    ttk1buf[:], ttk1buf[:], 0.0, None, op0=OP.max))
                op("sync", [("dve", dr1)],
                   lambda: nc.sync.dma_start(out=out[NS + 128:NS + TS, N:NT], in_=ttk1buf[:]),
                   "dout4", delta=16)

        # ---------- emit (waits embedded into the consuming instruction) ---
        with nc.Block(no_gpsimd_drain=True) as block:
            def make_body(engine_name):
                ops = plan[engine_name]

                # waits can be embedded only on single-instruction ops: a
                # matmul emits LDWEIGHTS before the MATMUL that would carry
                # the wait (weights read unguarded), and DMA triggers may
                # start descriptor generation early - both need standalone
                # event-sem waits ahead of the op
                embed_ok = engine_name in ("scalar", "vector", "tensor", "sync")

                def body(eng):
                    satisfied = {}
                    for waits, fn, inc in ops:
                        pend = []
                        for sem_name, val in waits:
                            if val is not None and satisfied.get(sem_name, -1) < val:
                                pend.append((sem_name, val))
                                satisfied[sem_name] = val
                        tail = pend[1:] if embed_ok else pend
                        for sem_name, val in tail:
                            eng.wait_ge(SEM[sem_name], val)
                        ins = fn()
                        if embed_ok and pend:
                            ins.wait_op(SEM[pend[0][0]], pend[0][1], "sem-ge")
                        if inc is None:
                            continue
                        if inc.startswith("din") or inc.startswith("dout"):
                            ins.then_inc(SEM[inc], 16)
                        else:
                            ins.then_inc(SEM[inc], 1)
                return body

            block.sync(make_body("sync"))
            block.tensor(make_body("tensor"))
            block.scalar(make_body("scalar"))
            block.vector(make_body("vector"))

    return nc


def _hilo(a):
    hi = a.astype(np.float16)
    lo = (a - hi.astype(np.float32)).astype(np.float16)
    return hi, lo


def build_in_maps(spatial_nodes, temporal_nodes, W_ss1, W_ss2, w_st, b_st, w_ts, b_ts):
    f = np.float32
    h16 = np.float16
    W12T = np.concatenate([W_ss1.T, W_ss2.T], axis=1).astype(f)
    W_hi, W_lo = _hilo(W12T)
    in_maps = []
    for c in range(N_CORES):
        b, hh = divmod(c, 2)
        tmask = (np.arange(T)[None, :] >= (hh * TS + np.arange(TS))[:, None]).astype(h16)
        # rotate spatial columns so this core's row-half sits at cols 0:NS
        spT = np.ascontiguousarray(np.roll(spatial_nodes[b].T, -hh * NS, axis=1), dtype=f)
        tmT = np.ascontiguousarray(temporal_nodes[b].T, dtype=f)
        sp_hi, sp_lo = _hilo(spT)
        tm_hi, tm_lo = _hilo(tmT)
        # block order [A=0:512, C=1024:1536, B=512:1024, D=1536:2048] for
        # hi then lo - matches the kernel's POS map and progressive DMAs
        blob1 = np.empty((D, B1_W), h16)
        blob1[:, 0:2 * D] = W_hi
        blob1[:, 2 * D:4 * D] = W_lo
        for p, s0 in enumerate((0, 1024, 512, 1536)):
            blob1[:, 4 * D + p * 512:4 * D + (p + 1) * 512] = sp_hi[:, s0:s0 + 512]
            blob1[:, 4 * D + N + p * 512:4 * D + N + (p + 1) * 512] = sp_lo[:, s0:s0 + 512]
        partsT = {
            "tmT_hi": tm_hi, "tmT_lo": tm_lo,
            "tmrT_hi": tm_hi[:, hh * TS:(hh + 1) * TS],
            "tmrT_lo": tm_lo[:, hh * TS:(hh + 1) * TS],
        }
        tmblob = np.empty((D, TM_W), h16)
        for nm, c0, c1 in TM_SLICES:
            tmblob[:, c0:c1] = partsT[nm]
        # host-side small linear transforms (same class as transpose/hi-lo prep)
        s1 = spT[:, 0:NS].T @ w_st[:D].astype(f)             # [NS]
        s2 = (temporal_nodes[b] @ w_st[D:].astype(f)) + f(b_st)   # [T]
        s1t = temporal_nodes[b, hh * TS:(hh + 1) * TS] @ w_ts[:D].astype(f)  # [TS]
        s2t = spT.T @ w_ts[D:].astype(f) + f(b_ts)           # [N] rotated order
        # k=1 packed: row t*8+blk holds s2t[blk*256 : blk*256+256]
        s2tbk1 = np.ascontiguousarray(s2t.astype(h16).reshape(8, 256)[
            np.tile(np.arange(8), 16), :])
        s2tbF = np.broadcast_to(s2t.astype(h16).ravel(), (128, N)).copy()
        biases = np.zeros((128, NBIAS), f)
        biases[:, 0:NCHUNK] = s1.reshape(NCHUNK, 128).T
        biases[0:128, NCHUNK] = s1t[0:128]
        biases[0:TS - 128, NCHUNK + 1] = s1t[128:TS]
        biases[:, NCHUNK + 2] = np.repeat(s1t[128:TS], 8)
        b_hi, b_lo = _hilo(biases)
        blob2s = np.empty((128, B2S_W), h16)
        blob2s[:, 0:T] = s2.astype(h16)[None, :]
        blob2s[:, T:T + 256] = s2tbk1
        blob2s[:, T + 256:T + 256 + NBIAS] = b_hi
        blob2s[:, T + 256 + NBIAS:] = b_lo
        in_maps.append({
            "blob1": blob1,
            "blob2s": blob2s,
            "s2tbF": s2tbF,
            "tmblob": tmblob,
            "ttmask": tmask,
        })
    return in_maps


def assemble(results):
    out = np.empty((B, NT, NT), np.float32)
    for c in range(N_CORES):
        b, h = divmod(c, 2)
        r = results[c]["out"].astype(np.float32)
        # un-rotate spatial columns (host rotated by -h*NS)
        sp_cols = np.roll(r[:, 0:N], h * NS, axis=1)
        out[b, h * NS:(h + 1) * NS, 0:N] = sp_cols[0:NS]
        out[b, h * NS:(h + 1) * NS, N:NT] = r[0:NS, N:NT]
        out[b, N + h * TS: N + (h + 1) * TS, 0:N] = sp_cols[NS:ROWS]
        out[b, N + h * TS: N + (h + 1) * TS, N:NT] = r[NS:ROWS, N:NT]
    return out


_NC = None


def kernel(**inputs):
    global _NC
    if _NC is None:
        _NC = build_program()
    in_maps = build_in_maps(**inputs)
    res = run_bass_kernel_spmd(_NC, in_maps, list(range(N_CORES)))
    return assemble(res.results)
